# revision 1
# baseline (speedup 1.0000x reference)
"""Trainium2 Bass kernel for nn_DiscreteModel (GNN message passing).

Strategy: shard by node rows across 8 cores (512 rows each). All per-node
tensors are kept feature-major ([feature, node]) on-chip so the contraction
dim of every matmul sits on SBUF partitions. The host pre-transposes the
od_mat shard and all weights, folds the random-walk projection W_rw and the
1/8 mean into the layer-1 weight block, and pads HID 2112 -> 2176.

Pipeline per core (n = 512 node columns):
  gather  : 32 indirect DMAs pull memory[walks] rows, tree-sum to Gs,
            PE-transpose to GsT -> bottom half of the "mixed" k-tile
            (top half = memT shard, DMA'd directly).
  layer1  : hiddenT[h] = relu(sum_k W1pT[k,h].T @ rawT[k] + b1), k = 32 od
            tiles (resident, 8MB) + 1 mixed tile; 17 h-tiles of 128.
  layer2  : msgT accumulates W2T[h].T @ hiddenT[h] in one PSUM bank.
  GRU     : gate matmuls K=64, gates via ACT sigmoid/tanh + DVE ops.
  predict : actT = relu(Wp1 @ updT + bp1); outT[m] = Wp2T[m].T @ actT + bp2.
All matmuls run in float32r (1 cycle/row at free dim 512, ~1e-4 rel err).
All DMAs use plain partition-major APs; every layout permutation happens on
the host (the Tile race tracker mis-handles partition-not-first dest APs).
"""

import numpy as np

import concourse.bass as bass
import concourse.bacc as bacc
import concourse.tile as tile
from concourse import mybir
from concourse.masks import make_identity

N = 4096        # nodes
MD = 64         # memory dim
MSG = 64        # message dim
WL = 8          # walk length
HID = 2112
HT = 17         # h-tiles (HID padded to 17*128 = 2176)
HIDP = HT * 128
NC = 8          # cores
R = N // NC     # rows (nodes) per core = 512
NT = R // 128   # node tiles per core = 4
KT = 33         # rawT k-tiles: 32 od + 1 mixed
F32 = mybir.dt.float32
F32R = mybir.dt.float32r
BF16 = mybir.dt.bfloat16
I32 = mybir.dt.int32

BF16_L1 = True   # layer-1 (od x W1) in bf16: halves the dominant DMA stream
_PROG = None


def _build_program():
    nc = bacc.Bacc("TRN2", target_bir_lowering=False, debug=False, num_devices=NC)

    # ---- DRAM I/O (all pre-laid-out on host, partition-major) ----
    mem_d = nc.dram_tensor("mem", [N, MD], F32, kind="ExternalInput").ap()
    L1DT = BF16 if BF16_L1 else F32R
    memT_d = nc.dram_tensor("memT", [MD, R], L1DT, kind="ExternalInput").ap()
    memf_d = nc.dram_tensor("memf", [MD, R], F32R, kind="ExternalInput").ap()
    od_d = nc.dram_tensor("odv", [128, 32 * R], L1DT, kind="ExternalInput").ap()
    walks_d = nc.dram_tensor("walks", [128, NT * WL], I32, kind="ExternalInput").ap()
    w1h_d = nc.dram_tensor("w1h", [HT, 128, 32 * 128], L1DT, kind="ExternalInput").ap()
    w1m_d = nc.dram_tensor("w1m", [128, HT * 128], L1DT, kind="ExternalInput").ap()
    w2t_d = nc.dram_tensor("w2t", [128, HT * MSG], F32R, kind="ExternalInput").ap()
    wi_d = nc.dram_tensor("wi", [MSG, 3 * MD], F32R, kind="ExternalInput").ap()
    wh_d = nc.dram_tensor("wh", [MD, 3 * MD], F32R, kind="ExternalInput").ap()
    wp1_d = nc.dram_tensor("wp1", [MD, MD], F32R, kind="ExternalInput").ap()
    wp2_d = nc.dram_tensor("wp2", [MD, N], F32R, kind="ExternalInput").ap()
    bias_d = nc.dram_tensor("biases", [128, 64], F32, kind="ExternalInput").ap()
    out_d = nc.dram_tensor("outm", [32, 128, R], F32, kind="ExternalOutput").ap()

    AF = mybir.ActivationFunctionType

    with tile.TileContext(nc) as tc:
        with (
            tc.tile_pool(name="consts", bufs=1) as consts,
            tc.tile_pool(name="w1p", bufs=3) as w1p,
            tc.tile_pool(name="gp", bufs=2) as gp,
            tc.tile_pool(name="hp", bufs=3) as hp,
            tc.tile_pool(name="gates", bufs=1) as gates,
            tc.tile_pool(name="ostg", bufs=3) as ostg,
            tc.tile_pool(name="pmm", bufs=6, space="PSUM") as pmm,
            tc.tile_pool(name="pacc", bufs=1, space="PSUM") as pacc,
        ):
            # ---- walks first: the gather chain is the longest head ----
            wk = consts.tile([128, NT * WL], I32, tag="wk")
            nc.gpsimd.dma_start(out=wk[:], in_=walks_d[:])

            # first od chunks lead the ACT queue: the h=0 matmul march is
            # paced by od arrival
            odres = consts.tile([128, 32 * R], L1DT, tag="odres")
            CH = 4 * R
            for c in range(4):
                H2 = 2 * R
                nc.scalar.dma_start(
                    out=odres[:, c * H2:(c + 1) * H2],
                    in_=od_d[:, c * H2:(c + 1) * H2],
                )

            # ---- walk gather: 32 indirect DMAs issued up front (the Q7
            #      descriptor chain takes ~45us); the tree-sum + transpose
            #      that consume them are emitted mid-h-loop so the PE stream
            #      is not blocked behind them ----
            gare = consts.tile([128, NT * WL * MD], F32, tag="gare")
            for t in range(NT):
                for j in range(WL):
                    o = (t * WL + j) * MD
                    nc.gpsimd.indirect_dma_start(
                        out=gare[:, o:o + MD],
                        out_offset=None,
                        in_=mem_d[:],
                        in_offset=bass.IndirectOffsetOnAxis(
                            ap=wk[:, t * WL + j:t * WL + j + 1], axis=0),
                    )

            ident = consts.tile([128, 128], F32, tag="ident")
            make_identity(nc, ident[:])
            biasp = consts.tile([128, 64], F32, tag="biasp")
            nc.scalar.dma_start(out=biasp[:], in_=bias_d[:])

            # mixed rawT k-tile: [0:64] = memT shard, [64:128] = GsT (walk sums)
            mixed = consts.tile([128, R], L1DT, tag="mixed")
            nc.scalar.dma_start(out=mixed[0:MD, :], in_=memT_d[:])
            memf = consts.tile([MD, R], F32R, tag="memf")
            nc.scalar.dma_start(out=memf[:], in_=memf_d[:])
            for c in range(2, 8):
                nc.scalar.dma_start(
                    out=odres[:, c * CH:(c + 1) * CH],
                    in_=od_d[:, c * CH:(c + 1) * CH],
                )

            def emit_mix_build():
                # The real gather chain takes ~55us on the Q7, but the tile
                # scheduler's cost model thinks it is fast and would schedule
                # these gather-dependent PE/DVE ops at the head of the engine
                # streams, stalling everything. tile_wait_until floors their
                # modeled time so the main matmuls schedule first.
                with tc.tile_wait_until(0.060):
                  for t in range(NT):
                    ga3 = gare[:, t * WL * MD:(t + 1) * WL * MD].rearrange(
                        "p (j d) -> p j d", j=WL)
                    m4 = gp.tile([128, 4 * MD], F32, tag="m4")
                    m43 = m4[:].rearrange("p (j d) -> p j d", j=4)
                    nc.vector.tensor_add(out=m43, in0=ga3[:, 0:4, :], in1=ga3[:, 4:8, :])
                    m2 = gp.tile([128, 2 * MD], F32, tag="m2")
                    m23 = m2[:].rearrange("p (j d) -> p j d", j=2)
                    nc.vector.tensor_add(out=m23, in0=m43[:, 0:2, :], in1=m43[:, 2:4, :])
                    m1 = gp.tile([128, MD], F32, tag="m1")
                    nc.vector.tensor_add(out=m1[:], in0=m2[:, 0:MD], in1=m2[:, MD:2 * MD])
                    tr = pacc.tile([MD, 128], F32, tag="pred")
                    nc.tensor.transpose(out=tr[:], in_=m1[:], identity=ident[:])
                    nc.vector.tensor_copy(
                        out=mixed[MD:128, t * 128:(t + 1) * 128], in_=tr[:])

            mixed_r = mixed[:]

            # ---- layer 1 + layer 2 accumulation ----
            ps_msg = pacc.tile([MSG, R], F32, tag="msg")
            DELAY = 5
            pending = []

            def finalize(h, ps):
                # mixed k-tile contribution arrives late (gather chain);
                # delaying it by DELAY h-groups keeps the PE dense.
                nc.tensor.matmul(
                    out=ps[:],
                    lhsT=w1m_sb[:, h * 128:(h + 1) * 128],
                    rhs=mixed_r,
                    start=False, stop=True,
                )
                hid = hp.tile([128, R], F32R, tag="hid")
                nc.scalar.activation(hid[:], ps[:], AF.Relu, bias=biasp[:, h:h + 1])
                nc.tensor.matmul(
                    out=ps_msg[:],
                    lhsT=w2t_sb[:, h * MSG:(h + 1) * MSG],
                    rhs=hid[:],
                    start=(h == 0), stop=(h == HT - 1),
                )

            for h in range(HT):
                if h == 2:
                    w1m_sb = consts.tile([128, HT * 128], L1DT, tag="w1m")
                    nc.scalar.dma_start(out=w1m_sb[:], in_=w1m_d[:])
                    w2t_sb = consts.tile([128, HT * MSG], F32R, tag="w2t")
                    nc.scalar.dma_start(out=w2t_sb[:], in_=w2t_d[:])
                    wh_sb = consts.tile([MD, 3 * MD], F32R, tag="wh")
                    nc.scalar.dma_start(out=wh_sb[:], in_=wh_d[:])
                if h == 3:
                    # h_n = memT @ Wh_n and its bias add depend only on memf;
                    # do them in the DMA-paced ramp where the PE has slack
                    ps_hn = pmm.tile([MD, R], F32, tag="mm")
                    nc.tensor.matmul(out=ps_hn[:], lhsT=wh_sb[:, 128:192],
                                     rhs=memf[:], start=True, stop=True)
                    hnb = gates.tile([MD, R], F32, tag="hnb")
                    nc.vector.tensor_scalar_add(out=hnb[:], in0=ps_hn[:],
                                                scalar1=biasp[0:MD, 20:21])
                if h == 4:
                    emit_mix_build()
                w1t = w1p.tile([128, 32 * 128], L1DT, tag="w1t")
                eng = nc.scalar if (h >= 2 and h % 4 == 3) else nc.sync
                if h < 2:
                    HKC = 16 * 128
                    eng.dma_start(out=w1t[:, :HKC], in_=w1h_d[h][:, :HKC])
                    eng.dma_start(out=w1t[:, HKC:], in_=w1h_d[h][:, HKC:])
                else:
                    eng.dma_start(out=w1t[:], in_=w1h_d[h])
                ps = pmm.tile([128, R], F32, tag="mm")
                for k in range(32):
                    nc.tensor.matmul(
                        out=ps[:],
                        lhsT=w1t[:, k * 128:(k + 1) * 128],
                        rhs=odres[:, k * R:(k + 1) * R],
                        start=(k == 0), stop=False,
                    )
                pending.append((h, ps))
                if len(pending) > DELAY:
                    finalize(*pending.pop(0))
                if h >= 12 and pending:
                    # gather is long done by now; drain eagerly so the tail
                    # does not bunch 5 finalizes after the last main group
                    finalize(*pending.pop(0))
            for h, ps in pending:
                finalize(h, ps)

            wi_sb = consts.tile([MSG, 3 * MD], F32R, tag="wi")
            nc.scalar.dma_start(out=wi_sb[:], in_=wi_d[:])
            wp1_sb = consts.tile([MD, MD], F32R, tag="wp1")
            nc.scalar.dma_start(out=wp1_sb[:], in_=wp1_d[:])
            wp2_sb = consts.tile([MD, N], F32R, tag="wp2")
            nc.scalar.dma_start(out=wp2_sb[:], in_=wp2_d[:])

            msg_sb = gates.tile([MSG, R], F32R, tag="msg_sb")
            nc.vector.tensor_scalar_add(
                out=msg_sb[:], in0=ps_msg[:], scalar1=biasp[0:MSG, 18:19])
            msg_r = msg_sb[:]
            memT_r = memf[:]

            # ---- GRU + prediction, column-split so the serial
            #      ACT/DVE chain pipelines across halves ----
            ps_rz = pmm.tile([128, R], F32, tag="mm")
            nc.tensor.matmul(out=ps_rz[:], lhsT=wi_sb[:, 0:128], rhs=msg_r,
                             start=True, stop=False)
            nc.tensor.matmul(out=ps_rz[:], lhsT=wh_sb[:, 0:128], rhs=memT_r,
                             start=False, stop=True)
            ps_in = pmm.tile([MD, R], F32, tag="mm")
            nc.tensor.matmul(out=ps_in[:], lhsT=wi_sb[:, 128:192], rhs=msg_r,
                             start=True, stop=True)
            rz = gates.tile([128, R], F32, tag="rz")
            rhn = gates.tile([MD, R], F32, tag="rhn")
            npre = gates.tile([MD, R], F32, tag="npre")
            n_t = gates.tile([MD, R], F32, tag="n_t")
            d_t = gates.tile([MD, R], F32, tag="d_t")
            z_t = gates.tile([MD, R], F32, tag="z_t")
            zd = gates.tile([MD, R], F32, tag="zd")
            upd = gates.tile([MD, R], F32R, tag="upd")
            ps_pred = pacc.tile([MD, R], F32, tag="pred")
            act = gates.tile([MD, R], F32R, tag="act")
            HR = R // 2
            for x in range(2):
                cs = slice(x * HR, (x + 1) * HR)
                nc.scalar.activation(rz[:, cs], ps_rz[:, cs], AF.Sigmoid,
                                     bias=biasp[:, 17:18])
                nc.vector.tensor_mul(out=rhn[:, cs], in0=rz[0:MD, cs], in1=hnb[:, cs])
                nc.vector.tensor_add(out=npre[:, cs], in0=ps_in[:, cs], in1=rhn[:, cs])
                nc.scalar.activation(n_t[:, cs], npre[:, cs], AF.Tanh,
                                     bias=biasp[0:MD, 19:20])
                nc.gpsimd.tensor_sub(out=d_t[:, cs], in0=memf[:, cs].bitcast(F32),
                                      in1=n_t[:, cs])
                nc.gpsimd.tensor_copy(out=z_t[:, cs], in_=rz[MD:128, cs])
                nc.vector.tensor_mul(out=zd[:, cs], in0=z_t[:, cs], in1=d_t[:, cs])
                nc.vector.tensor_add(out=upd[:, cs], in0=n_t[:, cs], in1=zd[:, cs])
                nc.tensor.matmul(out=ps_pred[:, cs], lhsT=wp1_sb[:], rhs=upd[:, cs],
                                 start=True, stop=True)
                nc.scalar.activation(act[:, cs], ps_pred[:, cs], AF.Relu,
                                     bias=biasp[0:MD, 21:22])
            act_r = act[:]

            for m in range(32):
                ps_o = pmm.tile([128, R], F32, tag="mm")
                nc.tensor.matmul(out=ps_o[:], lhsT=wp2_sb[:, m * 128:(m + 1) * 128],
                                 rhs=act_r, start=True, stop=True)
                if m % 4 == 0:
                    stage = ostg.tile([128, 4 * R], F32, tag="stage")
                nc.vector.tensor_scalar_add(
                    out=stage[:, (m % 4) * R:(m % 4 + 1) * R], in0=ps_o[:],
                    scalar1=biasp[:, 22 + m:23 + m])
                oeng = nc.sync if m % 2 == 0 else nc.scalar
                oeng.dma_start(
                    out=out_d[m],
                    in_=stage[:, (m % 4) * R:(m % 4 + 1) * R])

    nc.compile()
    return nc


def _get_program():
    global _PROG
    if _PROG is None:
        _PROG = _build_program()
    return _PROG


def _host_prep(memory, od_mat, walks, W_rw, b_rw, W1, b1, W2, b2,
               gru_Wi, gru_bi, gru_Wh, gru_bh, Wp1, bp1, Wp2, bp2):
    f = np.float32
    memory = np.ascontiguousarray(np.asarray(memory), dtype=f)
    od_mat = np.asarray(od_mat)
    walks = np.asarray(walks).astype(np.int32)
    W_rw = np.asarray(W_rw, dtype=f); b_rw = np.asarray(b_rw, dtype=f)
    W1 = np.asarray(W1, dtype=f); b1 = np.asarray(b1, dtype=f)
    W2 = np.asarray(W2, dtype=f); b2 = np.asarray(b2, dtype=f)
    gru_Wi = np.asarray(gru_Wi, dtype=f); gru_bi = np.asarray(gru_bi, dtype=f)
    gru_Wh = np.asarray(gru_Wh, dtype=f); gru_bh = np.asarray(gru_bh, dtype=f)
    Wp1 = np.asarray(Wp1, dtype=f); bp1 = np.asarray(bp1, dtype=f)
    Wp2 = np.asarray(Wp2, dtype=f); bp2 = np.asarray(bp2, dtype=f)

    # layer-1 weights, column-permuted to [od | dest | walk] with W_rw and the
    # 1/8 mean folded into the walk block; HID padded to 2176
    W1od = W1[:, MD:MD + N]
    W1dest = W1[:, 0:MD]
    W1rw = W1[:, MD + N:]
    W1g = (W1rw @ W_rw) / np.float32(8.0)
    W1p = np.concatenate([W1od, W1dest, W1g], axis=1)          # [2112, 4224]
    W1pT = np.zeros((KT * 128, HIDP), dtype=f)
    W1pT[:, :HID] = W1p.T
    # w1h[h][p, k*128+c] = W1pT[k*128+p, h*128+c] for the 32 od k-tiles;
    # the mixed k-tile (rows 4096:4224) is its own resident tensor w1m
    w1h = np.ascontiguousarray(
        W1pT[:32 * 128].reshape(32, 128, HT, 128)
        .transpose(2, 1, 0, 3).reshape(HT, 128, 32 * 128))
    w1m = np.ascontiguousarray(W1pT[32 * 128:])               # [128, 2176]

    b1p = np.zeros(HIDP, dtype=f)
    b1p[:HID] = b1 + W1rw @ b_rw

    W2tp = np.zeros((HIDP, MSG), dtype=f)
    W2tp[:HID] = W2.T
    # w2t[p, h*64+c] = W2tp[h*128+p, c]
    w2t = np.ascontiguousarray(
        W2tp.reshape(HT, 128, MSG).transpose(1, 0, 2).reshape(128, HT * MSG))

    def pad128(v):
        o = np.zeros(128, dtype=f)
        o[:v.shape[0]] = v
        return o

    # biases packed as [128 partitions, 64 columns]
    biases = np.zeros((64, 128), dtype=f)
    biases[0:HT] = b1p.reshape(HT, 128)
    biases[17] = gru_bi[:128] + gru_bh[:128]
    biases[18] = pad128(b2)
    biases[19] = pad128(gru_bi[128:])
    biases[20] = pad128(gru_bh[128:])
    biases[21] = pad128(bp1)
    biases[22:54] = bp2.reshape(32, 128)
    biases = np.ascontiguousarray(biases.T)                    # [128, 64]

    if BF16_L1:
        import ml_dtypes
        l1 = lambda a: np.ascontiguousarray(a.astype(ml_dtypes.bfloat16))
    else:
        l1 = lambda a: a
    w1h = l1(w1h)
    w1m = l1(w1m)
    shared = {
        "mem": memory,
        "w1h": w1h,
        "w1m": w1m,
        "w2t": w2t,
        "wi": np.ascontiguousarray(gru_Wi.T),
        "wh": np.ascontiguousarray(gru_Wh.T),
        "wp1": np.ascontiguousarray(Wp1.T),
        "wp2": np.ascontiguousarray(Wp2.T),
        "biases": biases,
    }
    in_maps = []
    for c in range(NC):
        sl = slice(c * R, (c + 1) * R)
        odc = np.asarray(od_mat[sl], dtype=f)
        # odv[p, k*R+n] = od[c*R+n, k*128+p]
        odv = np.ascontiguousarray(
            odc.T.reshape(32, 128, R).transpose(1, 0, 2).reshape(128, 32 * R))
        # walks[p, t*WL+j] = walks[c*R + t*128 + p, j]
        wkc = np.ascontiguousarray(
            walks[sl].reshape(NT, 128, WL).transpose(1, 0, 2).reshape(128, NT * WL))
        memT = np.ascontiguousarray(memory[sl].T)
        in_maps.append(dict(
            shared,
            memT=l1(memT),
            memf=memT,
            odv=l1(odv),
            walks=wkc,
        ))
    return in_maps


def _assemble(results):
    od = np.empty((N, N), dtype=np.float32)
    for c in range(NC):
        # outm[m, p, n] = od[c*R+n, m*128+p]
        od[c * R:(c + 1) * R, :] = (
            results[c]["outm"].transpose(2, 0, 1).reshape(R, N))
    return od


def _install_ntff_shim():
    """The agent image's antenv lacks axon_hooks, so trace=True dies on
    import. Recreate the module with the ctypes-based NTFF hook that
    trn_agent_boot would have registered."""
    import sys
    import types
    if "antenv.axon_hooks" in sys.modules:
        return
    from trn_agent_boot.trn_boot import _ntff_profile_via_ctypes
    hook = _ntff_profile_via_ctypes("/opt/axon/libaxon_pjrt.so")
    mod = types.ModuleType("antenv.axon_hooks")
    mod._hook = hook
    mod.get_axon_ntff_profile_hook = lambda: mod._hook
    mod.set_axon_ntff_profile_hook = lambda h: setattr(mod, "_hook", h)
    sys.modules["antenv.axon_hooks"] = mod


def run(inputs, trace=False):
    """Run on 8 NeuronCores; returns (od [N,N] f32, BassKernelResults)."""
    from concourse.bass_utils import run_bass_kernel_spmd
    if trace:
        try:
            _install_ntff_shim()
        except Exception as e:
            print(f"ntff shim failed ({e}); running without trace")
            trace = False
    nc = _get_program()
    in_maps = _host_prep(**inputs)
    res = run_bass_kernel_spmd(nc, in_maps, list(range(NC)), trace=trace)
    return _assemble(res.results), res


def kernel(**inputs):
    od, _ = run(inputs)
    return od



# revision 9
# speedup vs baseline: 1.3623x; 1.3623x over previous
"""Trainium2 Bass kernel for nn_DiscreteModel (GNN message passing).

Strategy: shard by node rows across 8 cores (512 rows each). All per-node
tensors are kept feature-major ([feature, node]) on-chip so the contraction
dim of every matmul sits on SBUF partitions. The host pre-transposes the
od_mat shard and all weights, folds the random-walk projection W_rw and the
1/8 mean into the layer-1 weight block, and pads HID 2112 -> 2176.

The dominant cost is the od x W1 layer-1 stream (4096 of the 4224 k dims).
It runs in fp8 e4m3 DoubleRow mode (2 k-subtiles per matmul, 2x PE rate):
od is mean-centered (od = 0.5 + odc, the exact rank-1 term 0.5*rowsum(W1od)
folds into b1) so fp8 quantization noise halves relative to signal, both
operands are scaled x16 and the x256 product scale rides through the relu
(relu(256x) = 256*relu(x)) and divides out of W2. Host-side fp8 rel err on
the final output: 5.8e-3 (gate 2e-2).

Pipeline per core (n = 512 node columns):
  gather  : one 4096-row indirect DMA pulls memory[walks], tree-sum to Gs,
            PE-transpose to GsT -> bottom half of the "mixed" k-tile
            (top half = memT shard, DMA'd directly). bf16, non-DoubleRow.
  layer1  : hiddenT[h] = relu(sum_k W1pT[k,h].T @ rawT[k] + b1), k = 16
            DoubleRow od pairs (resident, 4MB fp8) + 1 mixed tile; 17
            h-tiles of 128.
  layer2  : msgT accumulates W2T[h].T @ hiddenT[h] in one PSUM bank.
  GRU     : gate matmuls K=64, gates via ACT sigmoid/tanh + DVE ops.
  predict : actT = relu(Wp1 @ updT + bp1); outT[m] = Wp2xT[m].T @ actxT
            with bp2 folded in as a 65th contraction row against ones.
Non-fp8 matmuls run in float32r (1 cycle/row at free dim >= 256). Output
is written bf16 and upcast on the host.
"""

import os

import numpy as np

import concourse.bass as bass
import concourse.bacc as bacc
import concourse.tile as tile
from concourse import mybir
from concourse.masks import make_identity

N = 4096        # nodes
MD = 64         # memory dim
MSG = 64        # message dim
WL = 8          # walk length
HID = 2112
HT = 17         # h-tiles (HID padded to 17*128 = 2176)
HIDP = HT * 128
NC = 8          # cores
R = N // NC     # rows (nodes) per core = 512
NT = R // 128   # node tiles per core = 4
F32 = mybir.dt.float32
F32R = mybir.dt.float32r
BF16 = mybir.dt.bfloat16
FP8 = mybir.dt.float8e4
I32 = mybir.dt.int32

SC = 16.0       # per-operand fp8 scale; product scale 256 rides to W2
DELAY = 5       # h-groups the mixed k-tile matmul trails by (PSUM-bound <=5)
_PROG = None


def _build_program():
    nc = bacc.Bacc("TRN2", target_bir_lowering=False, debug=False, num_devices=NC)

    # ---- DRAM I/O (all pre-laid-out on host, partition-major) ----
    mem_d = nc.dram_tensor("mem", [N, MD], F32, kind="ExternalInput").ap()
    memT_d = nc.dram_tensor("memT", [MD, R], BF16, kind="ExternalInput").ap()
    memf_d = nc.dram_tensor("memf", [MD, R], F32R, kind="ExternalInput").ap()
    od_d = nc.dram_tensor("odv", [128, 32 * R], FP8, kind="ExternalInput").ap()
    walks_d = nc.dram_tensor("walks", [128, NT * WL], I32, kind="ExternalInput").ap()
    w1h_d = nc.dram_tensor("w1h", [HT, 128, 32 * 128], FP8, kind="ExternalInput").ap()
    w1m_d = nc.dram_tensor("w1m", [128, HT * 128], BF16, kind="ExternalInput").ap()
    w2t_d = nc.dram_tensor("w2t", [128, HT * MSG], F32R, kind="ExternalInput").ap()
    wi_d = nc.dram_tensor("wi", [MSG, 3 * MD], F32R, kind="ExternalInput").ap()
    wh_d = nc.dram_tensor("wh", [MD, 3 * MD], F32R, kind="ExternalInput").ap()
    wp1_d = nc.dram_tensor("wp1", [MD, MD], F32R, kind="ExternalInput").ap()
    wp2_d = nc.dram_tensor("wp2", [MD + 1, N], F32R, kind="ExternalInput").ap()
    bias_d = nc.dram_tensor("biases", [128, 64], F32, kind="ExternalInput").ap()
    ones_d = nc.dram_tensor("ones", [1, R], F32R, kind="ExternalInput").ap()
    out_d = nc.dram_tensor("outm", [32, 128, R], BF16, kind="ExternalOutput").ap()

    AF = mybir.ActivationFunctionType
    DR = mybir.MatmulPerfMode.DoubleRow

    with tile.TileContext(nc) as tc:
        with (
            tc.tile_pool(name="consts", bufs=1) as consts,
            tc.tile_pool(name="w1p", bufs=3) as w1p,
            tc.tile_pool(name="gp", bufs=2) as gp,
            tc.tile_pool(name="hp", bufs=3) as hp,
            tc.tile_pool(name="gates", bufs=1) as gates,
            tc.tile_pool(name="ostg", bufs=3) as ostg,
            tc.tile_pool(name="pmm", bufs=6, space="PSUM") as pmm,
            tc.tile_pool(name="pacc", bufs=1, space="PSUM") as pacc,
        ):
            # ---- walks first: the gather chain is the longest head ----
            wk = consts.tile([128, NT * WL], I32, tag="wk")
            nc.gpsimd.dma_start(out=wk[:], in_=walks_d[:])

            # first od chunks lead the DMA queues: the h=0 matmul march is
            # paced by od arrival. k-tile chunks: 2, 6, 8, 8, 8.
            odres = consts.tile([128, 32 * R], FP8, tag="odres")
            CHK = [(0, 2), (2, 6), (8, 8), (16, 8), (24, 8)]
            for c0, cn in CHK[:2]:
                nc.scalar.dma_start(
                    out=odres[:, c0 * R:(c0 + cn) * R],
                    in_=od_d[:, c0 * R:(c0 + cn) * R],
                )

            # ---- walk gather: one indirect DMA (4096 row descriptors; one
            #      instruction amortizes the gpsimd descriptor-gen startup) ----
            gare = consts.tile([128, NT * WL * MD], F32, tag="gare")
            if os.environ.get("KGATHER", "merged") == "merged":
                nc.gpsimd.indirect_dma_start(
                    out=gare[:],
                    out_offset=None,
                    in_=mem_d[:],
                    in_offset=bass.IndirectOffsetOnAxis(ap=wk[:], axis=0),
                )
            else:
                for t in range(NT):
                    for j in range(WL):
                        o = (t * WL + j) * MD
                        nc.gpsimd.indirect_dma_start(
                            out=gare[:, o:o + MD],
                            out_offset=None,
                            in_=mem_d[:],
                            in_offset=bass.IndirectOffsetOnAxis(
                                ap=wk[:, t * WL + j:t * WL + j + 1], axis=0),
                        )

            for c0, cn in CHK[2:]:
                nc.scalar.dma_start(
                    out=odres[:, c0 * R:(c0 + cn) * R],
                    in_=od_d[:, c0 * R:(c0 + cn) * R],
                )

            ident = consts.tile([128, 128], F32, tag="ident")
            make_identity(nc, ident[:])
            biasp = consts.tile([128, 64], F32, tag="biasp")
            nc.scalar.dma_start(out=biasp[:], in_=bias_d[:])

            # mixed rawT k-tile: [0:64] = memT shard, [64:128] = GsT (walk sums)
            mixed = consts.tile([128, R], BF16, tag="mixed")
            nc.scalar.dma_start(out=mixed[0:MD, :], in_=memT_d[:])
            memf = consts.tile([MD, R], F32R, tag="memf")
            nc.scalar.dma_start(out=memf[:], in_=memf_d[:])

            def emit_mix_build():
                # Floor the modeled time so the gather-dependent PE/DVE ops
                # schedule after a few h-groups instead of at the head of the
                # engine streams (the real gather takes longer than the tile
                # cost model thinks).
                with tc.tile_wait_until(0.012):
                  for t in range(NT):
                    ga3 = gare[:, t * WL * MD:(t + 1) * WL * MD].rearrange(
                        "p (j d) -> p j d", j=WL)
                    m4 = gp.tile([128, 4 * MD], F32, tag="m4")
                    m43 = m4[:].rearrange("p (j d) -> p j d", j=4)
                    nc.vector.tensor_add(out=m43, in0=ga3[:, 0:4, :], in1=ga3[:, 4:8, :])
                    m2 = gp.tile([128, 2 * MD], F32, tag="m2")
                    m23 = m2[:].rearrange("p (j d) -> p j d", j=2)
                    nc.vector.tensor_add(out=m23, in0=m43[:, 0:2, :], in1=m43[:, 2:4, :])
                    m1 = gp.tile([128, MD], F32, tag="m1")
                    nc.vector.tensor_add(out=m1[:], in0=m2[:, 0:MD], in1=m2[:, MD:2 * MD])
                    tr = pacc.tile([MD, 128], F32, tag="pred")
                    nc.tensor.transpose(out=tr[:], in_=m1[:], identity=ident[:])
                    nc.vector.tensor_copy(
                        out=mixed[MD:128, t * 128:(t + 1) * 128], in_=tr[:])

            mixed_r = mixed[:]
            odres3 = odres[:].rearrange("p (k n) -> p k n", k=32)

            # ---- layer 1 + layer 2 accumulation ----
            ps_msg = pacc.tile([MSG, R], F32, tag="msg")
            pending = []

            def finalize(h, ps):
                # mixed k-tile contribution arrives late (gather chain);
                # delaying it by DELAY h-groups keeps the PE dense.
                nc.tensor.matmul(
                    out=ps[:],
                    lhsT=w1m_sb[:, h * 128:(h + 1) * 128],
                    rhs=mixed_r,
                    start=False, stop=True,
                )
                hid = hp.tile([128, R], F32R, tag="hid")
                nc.scalar.activation(hid[:], ps[:], AF.Relu, bias=biasp[:, h:h + 1])
                nc.tensor.matmul(
                    out=ps_msg[:],
                    lhsT=w2t_sb[:, h * MSG:(h + 1) * MSG],
                    rhs=hid[:],
                    start=(h == 0), stop=(h == HT - 1),
                )

            for h in range(HT):
                if h == 2:
                    w1m_sb = consts.tile([128, HT * 128], BF16, tag="w1m")
                    nc.scalar.dma_start(out=w1m_sb[:], in_=w1m_d[:])
                    w2t_sb = consts.tile([128, HT * MSG], F32R, tag="w2t")
                    nc.scalar.dma_start(out=w2t_sb[:], in_=w2t_d[:])
                    wh_sb = consts.tile([MD, 3 * MD], F32R, tag="wh")
                    nc.scalar.dma_start(out=wh_sb[:], in_=wh_d[:])
                if h == 3:
                    # h_n = memT @ Wh_n and its bias add depend only on memf;
                    # do them in the DMA-paced ramp where the PE has slack
                    ps_hn = pmm.tile([MD, R], F32, tag="mm")
                    nc.tensor.matmul(out=ps_hn[:], lhsT=wh_sb[:, 128:192],
                                     rhs=memf[:], start=True, stop=True)
                    hnb = gates.tile([MD, R], F32, tag="hnb")
                    nc.vector.tensor_scalar_add(out=hnb[:], in0=ps_hn[:],
                                                scalar1=biasp[0:MD, 20:21])
                if h == 4:
                    emit_mix_build()
                w1t = w1p.tile([128, 32 * 128], FP8, tag="w1t")
                eng = nc.scalar if (h >= 2 and h % 4 == 3) else nc.sync
                if h == 0:
                    HKC = 16 * 128
                    eng.dma_start(out=w1t[:, :HKC], in_=w1h_d[h][:, :HKC])
                    eng.dma_start(out=w1t[:, HKC:], in_=w1h_d[h][:, HKC:])
                else:
                    eng.dma_start(out=w1t[:], in_=w1h_d[h])
                w1t3 = w1t[:].rearrange("p (k c) -> p k c", k=32)
                ps = pmm.tile([128, R], F32, tag="mm")
                for kk in range(16):
                    nc.tensor.matmul(
                        out=ps[:],
                        lhsT=w1t3[:, 2 * kk:2 * kk + 2, :],
                        rhs=odres3[:, 2 * kk:2 * kk + 2, :],
                        start=(kk == 0), stop=False,
                        perf_mode=DR,
                    )
                pending.append((h, ps))
                if len(pending) > DELAY:
                    finalize(*pending.pop(0))
                if h >= 12 and pending:
                    # gather is long done by now; drain eagerly so the tail
                    # does not bunch 5 finalizes after the last main group
                    finalize(*pending.pop(0))
            for h, ps in pending:
                finalize(h, ps)

            wi_sb = consts.tile([MSG, 3 * MD], F32R, tag="wi")
            nc.scalar.dma_start(out=wi_sb[:], in_=wi_d[:])
            wp1_sb = consts.tile([MD, MD], F32R, tag="wp1")
            nc.scalar.dma_start(out=wp1_sb[:], in_=wp1_d[:])
            wp2_sb = consts.tile([MD + 1, N], F32R, tag="wp2")
            nc.scalar.dma_start(out=wp2_sb[:], in_=wp2_d[:])

            msg_sb = gates.tile([MSG, R], F32R, tag="msg_sb")
            nc.vector.tensor_scalar_add(
                out=msg_sb[:], in0=ps_msg[:], scalar1=biasp[0:MSG, 18:19])
            msg_r = msg_sb[:]
            memT_r = memf[:]

            # ---- GRU + prediction, column-split so the serial
            #      ACT/DVE chain pipelines across halves ----
            ps_rz = pmm.tile([128, R], F32, tag="mm")
            nc.tensor.matmul(out=ps_rz[:], lhsT=wi_sb[:, 0:128], rhs=msg_r,
                             start=True, stop=False)
            nc.tensor.matmul(out=ps_rz[:], lhsT=wh_sb[:, 0:128], rhs=memT_r,
                             start=False, stop=True)
            ps_in = pmm.tile([MD, R], F32, tag="mm")
            nc.tensor.matmul(out=ps_in[:], lhsT=wi_sb[:, 128:192], rhs=msg_r,
                             start=True, stop=True)
            rz = gates.tile([128, R], F32, tag="rz")
            rhn = gates.tile([MD, R], F32, tag="rhn")
            npre = gates.tile([MD, R], F32, tag="npre")
            n_t = gates.tile([MD, R], F32, tag="n_t")
            d_t = gates.tile([MD, R], F32, tag="d_t")
            z_t = gates.tile([MD, R], F32, tag="z_t")
            zd = gates.tile([MD, R], F32, tag="zd")
            # act rows 0:64 = relu(pred1); row 64 = ones so the out matmul's
            # 65th contraction row adds bp2 (folded into wp2 row 64)
            actx = gates.tile([MD + 1, R], F32R, tag="actx")
            nc.scalar.dma_start(out=actx[MD:MD + 1, :], in_=ones_d[:])
            upd = gates.tile([MD, R], F32R, tag="upd")
            ps_pred = pacc.tile([MD, R], F32, tag="pred")
            HR = R // 2
            for x in range(2):
                cs = slice(x * HR, (x + 1) * HR)
                nc.scalar.activation(rz[:, cs], ps_rz[:, cs], AF.Sigmoid,
                                     bias=biasp[:, 17:18])
                nc.vector.tensor_mul(out=rhn[:, cs], in0=rz[0:MD, cs], in1=hnb[:, cs])
                nc.vector.tensor_add(out=npre[:, cs], in0=ps_in[:, cs], in1=rhn[:, cs])
                nc.scalar.activation(n_t[:, cs], npre[:, cs], AF.Tanh,
                                     bias=biasp[0:MD, 19:20])
                nc.gpsimd.tensor_sub(out=d_t[:, cs], in0=memf[:, cs].bitcast(F32),
                                      in1=n_t[:, cs])
                nc.gpsimd.tensor_copy(out=z_t[:, cs], in_=rz[MD:128, cs])
                nc.vector.tensor_mul(out=zd[:, cs], in0=z_t[:, cs], in1=d_t[:, cs])
                nc.vector.tensor_add(out=upd[:, cs], in0=n_t[:, cs], in1=zd[:, cs])
                nc.tensor.matmul(out=ps_pred[:, cs], lhsT=wp1_sb[:], rhs=upd[:, cs],
                                 start=True, stop=True)
                nc.scalar.activation(actx[0:MD, cs], ps_pred[:, cs], AF.Relu,
                                     bias=biasp[0:MD, 21:22])
            act_r = actx[:]

            for m in range(32):
                ps_o = pmm.tile([128, R], F32, tag="mm")
                nc.tensor.matmul(out=ps_o[:], lhsT=wp2_sb[:, m * 128:(m + 1) * 128],
                                 rhs=act_r, start=True, stop=True)
                if m % 4 == 0:
                    stage = ostg.tile([128, 4 * R], BF16, tag="stage")
                dst = stage[:, (m % 4) * R:(m % 4 + 1) * R]
                # alternate PSUM->SBUF bf16 casts across DVE and ACT so
                # neither engine serializes the tail
                if m % 2 == 0:
                    nc.vector.tensor_copy(out=dst, in_=ps_o[:])
                else:
                    nc.scalar.activation(dst, ps_o[:], AF.Copy)
                oeng = nc.sync if m % 2 == 0 else nc.gpsimd
                oeng.dma_start(
                    out=out_d[m],
                    in_=stage[:, (m % 4) * R:(m % 4 + 1) * R])

    nc.compile()
    return nc


def _get_program():
    global _PROG
    if _PROG is None:
        _PROG = _build_program()
    return _PROG


def _host_prep(memory, od_mat, walks, W_rw, b_rw, W1, b1, W2, b2,
               gru_Wi, gru_bi, gru_Wh, gru_bh, Wp1, bp1, Wp2, bp2):
    import ml_dtypes
    f = np.float32
    E4 = ml_dtypes.float8_e4m3
    memory = np.ascontiguousarray(np.asarray(memory), dtype=f)
    od_mat = np.asarray(od_mat)
    walks = np.asarray(walks).astype(np.int32)
    W_rw = np.asarray(W_rw, dtype=f); b_rw = np.asarray(b_rw, dtype=f)
    W1 = np.asarray(W1, dtype=f); b1 = np.asarray(b1, dtype=f)
    W2 = np.asarray(W2, dtype=f); b2 = np.asarray(b2, dtype=f)
    gru_Wi = np.asarray(gru_Wi, dtype=f); gru_bi = np.asarray(gru_bi, dtype=f)
    gru_Wh = np.asarray(gru_Wh, dtype=f); gru_bh = np.asarray(gru_bh, dtype=f)
    Wp1 = np.asarray(Wp1, dtype=f); bp1 = np.asarray(bp1, dtype=f)
    Wp2 = np.asarray(Wp2, dtype=f); bp2 = np.asarray(bp2, dtype=f)

    W1dest = W1[:, 0:MD]
    W1od = W1[:, MD:MD + N]
    W1rw = W1[:, MD + N:]
    W1g = (W1rw @ W_rw) / np.float32(8.0)

    # od k-tiles: fp8(16 * W1od.T), k-subtile-major for DoubleRow pairs
    W1odT = np.zeros((32 * 128, HIDP), dtype=f)
    W1odT[:, :HID] = W1od.T * np.float32(SC)
    w1h = np.ascontiguousarray(
        W1odT.reshape(32, 128, HT, 128)
        .transpose(2, 1, 0, 3).reshape(HT, 128, 32 * 128).astype(E4))

    # mixed k-tile weights [dest | walk], x256 to match the fp8 product scale
    W1m = np.concatenate([W1dest, W1g], axis=1) * np.float32(SC * SC)  # [2112,128]
    W1mT = np.zeros((128, HIDP), dtype=f)
    W1mT[:, :HID] = W1m.T
    w1m = np.ascontiguousarray(W1mT.astype(ml_dtypes.bfloat16))

    # b1 fold: rw bias + the exact od mean-centering rank-1 term, x256
    b1p = np.zeros(HIDP, dtype=f)
    b1p[:HID] = (b1 + W1rw @ b_rw + np.float32(0.5) * W1od.sum(axis=1)) \
        * np.float32(SC * SC)

    # W2 absorbs the 1/256 descale
    W2tp = np.zeros((HIDP, MSG), dtype=f)
    W2tp[:HID] = W2.T / np.float32(SC * SC)
    w2t = np.ascontiguousarray(
        W2tp.reshape(HT, 128, MSG).transpose(1, 0, 2).reshape(128, HT * MSG))

    # prediction: fold bp2 into wp2 as a 65th contraction row
    wp2x = np.concatenate(
        [np.ascontiguousarray(Wp2.T), bp2[None, :]], axis=0)  # [65, N]

    def pad128(v):
        o = np.zeros(128, dtype=f)
        o[:v.shape[0]] = v
        return o

    # biases packed as [128 partitions, 64 columns]
    biases = np.zeros((64, 128), dtype=f)
    biases[0:HT] = b1p.reshape(HT, 128)
    biases[17] = gru_bi[:128] + gru_bh[:128]
    biases[18] = pad128(b2)
    biases[19] = pad128(gru_bi[128:])
    biases[20] = pad128(gru_bh[128:])
    biases[21] = pad128(bp1)
    biases = np.ascontiguousarray(biases.T)                    # [128, 64]

    shared = {
        "mem": memory,
        "w1h": w1h,
        "w1m": w1m,
        "w2t": w2t,
        "wi": np.ascontiguousarray(gru_Wi.T),
        "wh": np.ascontiguousarray(gru_Wh.T),
        "wp1": np.ascontiguousarray(Wp1.T),
        "wp2": np.ascontiguousarray(wp2x),
        "biases": biases,
        "ones": np.ones((1, R), dtype=f),
    }
    in_maps = []
    for c in range(NC):
        sl = slice(c * R, (c + 1) * R)
        odc = np.asarray(od_mat[sl], dtype=f)
        # odv[p, k*R+n] = 16*(od[c*R+n, k*128+p] - 0.5), fp8
        odv = np.ascontiguousarray(
            ((odc.T - np.float32(0.5)) * np.float32(SC))
            .reshape(32, 128, R).transpose(1, 0, 2).reshape(128, 32 * R)
            .astype(E4))
        # walks[p, t*WL+j] = walks[c*R + t*128 + p, j]
        wkc = np.ascontiguousarray(
            walks[sl].reshape(NT, 128, WL).transpose(1, 0, 2).reshape(128, NT * WL))
        memT = np.ascontiguousarray(memory[sl].T)
        in_maps.append(dict(
            shared,
            memT=np.ascontiguousarray(memT.astype(ml_dtypes.bfloat16)),
            memf=memT,
            odv=odv,
            walks=wkc,
        ))
    return in_maps


def _assemble(results):
    od = np.empty((N, N), dtype=np.float32)
    for c in range(NC):
        # outm[m, p, n] = od[c*R+n, m*128+p]
        od[c * R:(c + 1) * R, :] = (
            results[c]["outm"].astype(np.float32).transpose(2, 0, 1).reshape(R, N))
    return od


def _install_ntff_shim():
    """The agent image's antenv lacks axon_hooks, so trace=True dies on
    import. Recreate the module with the ctypes-based NTFF hook that
    trn_agent_boot would have registered."""
    import sys
    import types
    if "antenv.axon_hooks" in sys.modules:
        return
    from trn_agent_boot.trn_boot import _ntff_profile_via_ctypes
    hook = _ntff_profile_via_ctypes("/opt/axon/libaxon_pjrt.so")
    mod = types.ModuleType("antenv.axon_hooks")
    mod._hook = hook
    mod.get_axon_ntff_profile_hook = lambda: mod._hook
    mod.set_axon_ntff_profile_hook = lambda h: setattr(mod, "_hook", h)
    sys.modules["antenv.axon_hooks"] = mod


def run(inputs, trace=False):
    """Run on 8 NeuronCores; returns (od [N,N] f32, BassKernelResults)."""
    from concourse.bass_utils import run_bass_kernel_spmd
    if trace:
        try:
            _install_ntff_shim()
        except Exception as e:
            print(f"ntff shim failed ({e}); running without trace")
            trace = False
    nc = _get_program()
    in_maps = _host_prep(**inputs)
    res = run_bass_kernel_spmd(nc, in_maps, list(range(NC)), trace=trace)
    return _assemble(res.results), res


def kernel(**inputs):
    od, _ = run(inputs)
    return od


# revision 20
# speedup vs baseline: 1.3782x; 1.0117x over previous
"""Trainium2 Bass kernel for nn_DiscreteModel (GNN message passing).

Strategy: shard by node rows across 8 cores (512 rows each). All per-node
tensors are kept feature-major ([feature, node]) on-chip so the contraction
dim of every matmul sits on SBUF partitions. The host pre-transposes the
od_mat shard and all weights, folds the random-walk projection W_rw and the
1/8 mean into the layer-1 weight block, and pads HID 2112 -> 2176.

The dominant cost is the od x W1 layer-1 stream (4096 of the 4224 k dims).
It runs in fp8 e4m3 DoubleRow mode (2 k-subtiles per matmul, 2x PE rate):
od is mean-centered (od = 0.5 + odc, the exact rank-1 term 0.5*rowsum(W1od)
folds into b1) so fp8 quantization noise halves relative to signal, both
operands are scaled x16 and the x256 product scale rides through the relu
(relu(256x) = 256*relu(x)) and divides out of W2. Host-side fp8 rel err on
the final output: 5.8e-3 (gate 2e-2).

The walk gather is a matmul: the host one-hot-encodes walks into the count
matrix A[m, n] = #{j: walks[n, j] = m} (exact small ints in fp8), and
GsT = sum_k mem_k.T @ A_k runs as 16 DoubleRow matmuls (~3.5us) as soon as
A lands. This replaces 32 indirect DMAs whose descriptor generation
serialized ~35us on the gpsimd engine (hw reads exactly ONE offset per
partition per indirect DMA instruction, so they cannot be merged).

Pipeline per core (n = 512 node columns):
  gather  : GsT = mem.T @ A via DoubleRow fp8 -> bottom half of the "mixed"
            k-tile (top half = memT shard, DMA'd directly). Mixed is bf16.
  layer1  : hiddenT[h] = relu(sum_k W1pT[k,h].T @ rawT[k] + b1), k = 16
            DoubleRow od pairs (resident, 2MB fp8) + 1 mixed tile; 17
            h-tiles of 128.
  layer2  : msgT accumulates W2T[h].T @ hiddenT[h] in one PSUM bank.
  GRU     : gate matmuls K=64 at half width, gates via ACT sigmoid/tanh +
            DVE/gpsimd ops in quarter-width chunks for pipeline depth.
  predict : actT = relu(Wp1 @ updT + bp1); outT[m] = Wp2xT[m].T @ actxT
            with bp2 folded in as a 65th contraction row against ones.
Non-fp8 matmuls run in float32r (1 cycle/row at free dim >= 256). Output
is written bf16 (4 m-tiles per DMA) and upcast on the host.
"""

import numpy as np

import concourse.bass as bass
import concourse.bacc as bacc
import concourse.tile as tile
from concourse import mybir

N = 4096        # nodes
MD = 64         # memory dim
MSG = 64        # message dim
WL = 8          # walk length
HID = 2112
HT = 17         # h-tiles (HID padded to 17*128 = 2176)
HIDP = HT * 128
NC = 8          # cores
R = N // NC     # rows (nodes) per core = 512
NT = R // 128   # node tiles per core = 4
F32 = mybir.dt.float32
F32R = mybir.dt.float32r
BF16 = mybir.dt.bfloat16
FP8 = mybir.dt.float8e4
I32 = mybir.dt.int32

SC = 16.0       # per-operand fp8 scale; product scale 256 rides to W2
DELAY = 5       # h-groups the mixed k-tile matmul trails by (PSUM-bound <=5)
_PROG = None


def _build_program():
    nc = bacc.Bacc("TRN2", target_bir_lowering=False, debug=False, num_devices=NC)

    # ---- DRAM I/O (all pre-laid-out on host, partition-major) ----
    memT_d = nc.dram_tensor("memT", [MD, R], BF16, kind="ExternalInput").ap()
    memf_d = nc.dram_tensor("memf", [MD, R], F32R, kind="ExternalInput").ap()
    od_d = nc.dram_tensor("odv", [128, 32 * R], FP8, kind="ExternalInput").ap()
    aw_d = nc.dram_tensor("aw", [128, 32 * R], FP8, kind="ExternalInput").ap()
    mt8_d = nc.dram_tensor("mt8", [128, 32 * MD], FP8, kind="ExternalInput").ap()
    w1h_d = nc.dram_tensor("w1h", [HT, 128, 32 * 128], FP8, kind="ExternalInput").ap()
    w1m_d = nc.dram_tensor("w1m", [128, HT * 128], BF16, kind="ExternalInput").ap()
    w2t_d = nc.dram_tensor("w2t", [128, HT * MSG], F32R, kind="ExternalInput").ap()
    wi_d = nc.dram_tensor("wi", [MSG, 3 * MD], F32R, kind="ExternalInput").ap()
    wh_d = nc.dram_tensor("wh", [MD, 3 * MD], F32R, kind="ExternalInput").ap()
    wp1_d = nc.dram_tensor("wp1", [MD, MD], F32R, kind="ExternalInput").ap()
    wp2_d = nc.dram_tensor("wp2", [MD + 1, N], F32R, kind="ExternalInput").ap()
    bias_d = nc.dram_tensor("biases", [128, 64], F32, kind="ExternalInput").ap()
    ones_d = nc.dram_tensor("ones", [1, R], F32R, kind="ExternalInput").ap()
    out_d = nc.dram_tensor("outm", [128, 32 * R], BF16, kind="ExternalOutput").ap()

    AF = mybir.ActivationFunctionType
    DR = mybir.MatmulPerfMode.DoubleRow

    with tile.TileContext(nc) as tc:
        with (
            tc.tile_pool(name="consts", bufs=1) as consts,
            tc.tile_pool(name="w1p", bufs=3) as w1p,
            tc.tile_pool(name="hp", bufs=3) as hp,
            tc.tile_pool(name="gates", bufs=1) as gates,
            tc.tile_pool(name="ostg", bufs=3) as ostg,
            tc.tile_pool(name="pmm", bufs=6, space="PSUM") as pmm,
            tc.tile_pool(name="pacc", bufs=1, space="PSUM") as pacc,
        ):
            # ---- od shard first: the h=0 matmul march is paced by od
            #      arrival. Split the chunks across the sync and gpsimd
            #      queues (DMA issue is only possible from sync/scalar/
            #      gpsimd); the w1h h=0 tile leads the sync queue. ----
            odres = consts.tile([128, 32 * R], FP8, tag="odres")
            nc.sync.dma_start(out=odres[:, 0:2 * R], in_=od_d[:, 0:2 * R])
            w1t0 = w1p.tile([128, 32 * 128], FP8, tag="w1t")
            HKC = 16 * 128
            nc.sync.dma_start(out=w1t0[:, :HKC], in_=w1h_d[0][:, :HKC])
            nc.sync.dma_start(out=w1t0[:, HKC:], in_=w1h_d[0][:, HKC:])
            for c0, cn in [(2, 6), (8, 8)]:
                nc.sync.dma_start(
                    out=odres[:, c0 * R:(c0 + cn) * R],
                    in_=od_d[:, c0 * R:(c0 + cn) * R],
                )
            for c0, cn in [(16, 8), (24, 8)]:
                nc.gpsimd.dma_start(
                    out=odres[:, c0 * R:(c0 + cn) * R],
                    in_=od_d[:, c0 * R:(c0 + cn) * R],
                )

            # walk count matrix + fp8 memory for the matmul gather
            mt8 = consts.tile([128, 32 * MD], FP8, tag="mt8")
            nc.gpsimd.dma_start(out=mt8[:], in_=mt8_d[:])
            aw = consts.tile([128, 32 * R], FP8, tag="aw")
            for c in range(4):
                nc.gpsimd.dma_start(
                    out=aw[:, c * 8 * R:(c + 1) * 8 * R],
                    in_=aw_d[:, c * 8 * R:(c + 1) * 8 * R])

            biasp = consts.tile([128, 64], F32, tag="biasp")
            nc.scalar.dma_start(out=biasp[:], in_=bias_d[:])

            # mixed rawT k-tile: [0:64] = memT shard, [64:128] = GsT (walk sums)
            mixed = consts.tile([128, R], BF16, tag="mixed")
            nc.scalar.dma_start(out=mixed[0:MD, :], in_=memT_d[:])
            memf = consts.tile([MD, R], F32R, tag="memf")
            nc.scalar.dma_start(out=memf[:], in_=memf_d[:])

            mixed_r = mixed[:]
            odres3 = odres[:].rearrange("p (k n) -> p k n", k=32)
            aw3 = aw[:].rearrange("p (k n) -> p k n", k=32)
            mt3 = mt8[:].rearrange("p (k d) -> p k d", k=32)

            # ---- layer 1 + layer 2 accumulation ----
            ps_msg = pacc.tile([MSG, R], F32, tag="msg")
            pending = []

            def finalize(h, ps):
                # mixed k-tile contribution arrives late (gather matmuls);
                # delaying it by DELAY h-groups keeps the PE dense.
                nc.tensor.matmul(
                    out=ps[:],
                    lhsT=w1m_sb[:, h * 128:(h + 1) * 128],
                    rhs=mixed_r,
                    start=False, stop=True,
                )
                hid = hp.tile([128, R], F32R, tag="hid")
                nc.scalar.activation(hid[:], ps[:], AF.Relu, bias=biasp[:, h:h + 1])
                nc.tensor.matmul(
                    out=ps_msg[:],
                    lhsT=w2t_sb[:, h * MSG:(h + 1) * MSG],
                    rhs=hid[:],
                    start=(h == 0), stop=(h == HT - 1),
                )

            for h in range(HT):
                if h == 2:
                    w1m_sb = consts.tile([128, HT * 128], BF16, tag="w1m")
                    nc.scalar.dma_start(out=w1m_sb[:], in_=w1m_d[:])
                    w2t_sb = consts.tile([128, HT * MSG], F32R, tag="w2t")
                    nc.scalar.dma_start(out=w2t_sb[:], in_=w2t_d[:])
                    wh_sb = consts.tile([MD, 3 * MD], F32R, tag="wh")
                    nc.scalar.dma_start(out=wh_sb[:], in_=wh_d[:])
                if h == 3:
                    # h_n = memT @ Wh_n and its bias add depend only on memf;
                    # do them in the DMA-paced ramp where the PE has slack
                    ps_hn = pmm.tile([MD, R], F32, tag="mm")
                    nc.tensor.matmul(out=ps_hn[:], lhsT=wh_sb[:, 128:192],
                                     rhs=memf[:], start=True, stop=True)
                    hnb = gates.tile([MD, R], F32, tag="hnb")
                    nc.vector.tensor_scalar_add(out=hnb[:], in0=ps_hn[:],
                                                scalar1=biasp[0:MD, 20:21])
                    # walk gather on the PE: GsT = sum_k mem_k.T @ A_k
                    gsps = pacc.tile([MD, R], F32, tag="pred")
                    for kk in range(16):
                        nc.tensor.matmul(
                            out=gsps[:],
                            lhsT=mt3[:, 2 * kk:2 * kk + 2, :],
                            rhs=aw3[:, 2 * kk:2 * kk + 2, :],
                            start=(kk == 0), stop=(kk == 15),
                            perf_mode=DR,
                        )
                    nc.vector.tensor_copy(out=mixed[MD:128, :], in_=gsps[:])
                if h == 0:
                    w1t = w1t0
                else:
                    w1t = w1p.tile([128, 32 * 128], FP8, tag="w1t")
                    eng = nc.scalar if h % 4 == 3 else nc.sync
                    eng.dma_start(out=w1t[:], in_=w1h_d[h])
                w1t3 = w1t[:].rearrange("p (k c) -> p k c", k=32)
                ps = pmm.tile([128, R], F32, tag="mm")
                for kk in range(16):
                    nc.tensor.matmul(
                        out=ps[:],
                        lhsT=w1t3[:, 2 * kk:2 * kk + 2, :],
                        rhs=odres3[:, 2 * kk:2 * kk + 2, :],
                        start=(kk == 0), stop=False,
                        perf_mode=DR,
                    )
                pending.append((h, ps))
                if len(pending) > DELAY:
                    finalize(*pending.pop(0))
                if h >= 12 and pending:
                    # the gather is long done by now; drain eagerly so the
                    # tail does not bunch 5 finalizes after the last group
                    finalize(*pending.pop(0))
            for h, ps in pending:
                finalize(h, ps)

            wi_sb = consts.tile([MSG, 3 * MD], F32R, tag="wi")
            nc.scalar.dma_start(out=wi_sb[:], in_=wi_d[:])
            wp1_sb = consts.tile([MD, MD], F32R, tag="wp1")
            nc.scalar.dma_start(out=wp1_sb[:], in_=wp1_d[:])
            wp2_sb = consts.tile([MD + 1, N], F32R, tag="wp2")
            nc.scalar.dma_start(out=wp2_sb[:], in_=wp2_d[:])

            msg_sb = gates.tile([MSG, R], F32R, tag="msg_sb")
            nc.vector.tensor_scalar_add(
                out=msg_sb[:], in0=ps_msg[:], scalar1=biasp[0:MSG, 18:19])
            msg_r = msg_sb[:]
            memT_r = memf[:]

            # ---- GRU + prediction. Matmuls run at half width (f32r needs
            #      free >= 256); the serial ACT/DVE/gpsimd gate chain runs in
            #      quarter-width chunks so it pipelines deeper. ----
            ps_rz = pmm.tile([128, R], F32, tag="mm")
            nc.tensor.matmul(out=ps_rz[:], lhsT=wi_sb[:, 0:128], rhs=msg_r,
                             start=True, stop=False)
            nc.tensor.matmul(out=ps_rz[:], lhsT=wh_sb[:, 0:128], rhs=memT_r,
                             start=False, stop=True)
            ps_in = pmm.tile([MD, R], F32, tag="mm")
            nc.tensor.matmul(out=ps_in[:], lhsT=wi_sb[:, 128:192], rhs=msg_r,
                             start=True, stop=True)
            rz = gates.tile([128, R], F32, tag="rz")
            rhn = gates.tile([MD, R], F32, tag="rhn")
            npre = gates.tile([MD, R], F32, tag="npre")
            n_t = gates.tile([MD, R], F32, tag="n_t")
            d_t = gates.tile([MD, R], F32, tag="d_t")
            z_t = gates.tile([MD, R], F32, tag="z_t")
            zd = gates.tile([MD, R], F32, tag="zd")
            # actx rows 0:64 = relu(pred1); row 64 = ones so the out matmul's
            # 65th contraction row adds bp2 (folded into wp2 row 64)
            actx = gates.tile([MD + 1, R], F32R, tag="actx")
            nc.scalar.dma_start(out=actx[MD:MD + 1, :], in_=ones_d[:])
            upd = gates.tile([MD, R], F32R, tag="upd")
            ps_pred = pacc.tile([MD, R], F32, tag="pred")
            QR = R // 4
            for x in range(4):
                cs = slice(x * QR, (x + 1) * QR)
                nc.scalar.activation(rz[:, cs], ps_rz[:, cs], AF.Sigmoid,
                                     bias=biasp[:, 17:18])
                nc.vector.tensor_mul(out=rhn[:, cs], in0=rz[0:MD, cs], in1=hnb[:, cs])
                nc.vector.tensor_add(out=npre[:, cs], in0=ps_in[:, cs], in1=rhn[:, cs])
                nc.scalar.activation(n_t[:, cs], npre[:, cs], AF.Tanh,
                                     bias=biasp[0:MD, 19:20])
                nc.gpsimd.tensor_sub(out=d_t[:, cs], in0=memf[:, cs].bitcast(F32),
                                      in1=n_t[:, cs])
                nc.gpsimd.tensor_copy(out=z_t[:, cs], in_=rz[MD:128, cs])
                nc.vector.tensor_mul(out=zd[:, cs], in0=z_t[:, cs], in1=d_t[:, cs])
                nc.vector.tensor_add(out=upd[:, cs], in0=n_t[:, cs], in1=zd[:, cs])
                if x % 2 == 1:
                    hs = slice((x - 1) * QR, (x + 1) * QR)
                    nc.tensor.matmul(out=ps_pred[:, hs], lhsT=wp1_sb[:],
                                     rhs=upd[:, hs], start=True, stop=True)
                    nc.scalar.activation(actx[0:MD, hs], ps_pred[:, hs], AF.Relu,
                                         bias=biasp[0:MD, 21:22])
            act_r = actx[:]

            for m in range(32):
                ps_o = pmm.tile([128, R], F32, tag="mm")
                nc.tensor.matmul(out=ps_o[:], lhsT=wp2_sb[:, m * 128:(m + 1) * 128],
                                 rhs=act_r, start=True, stop=True)
                if m % 4 == 0:
                    stage = ostg.tile([128, 4 * R], BF16, tag="stage")
                dst = stage[:, (m % 4) * R:(m % 4 + 1) * R]
                # alternate PSUM->SBUF bf16 casts across DVE and ACT (gpsimd
                # cannot read PSUM) so neither engine serializes the tail
                if m % 2 == 0:
                    nc.vector.tensor_copy(out=dst, in_=ps_o[:])
                else:
                    nc.scalar.activation(dst, ps_o[:], AF.Copy)
                if m % 4 == 3:
                    nc.sync.dma_start(
                        out=out_d[:, (m - 3) * R:(m + 1) * R], in_=stage[:])

    nc.compile()
    return nc


def _get_program():
    global _PROG
    if _PROG is None:
        _PROG = _build_program()
    return _PROG


def _host_prep(memory, od_mat, walks, W_rw, b_rw, W1, b1, W2, b2,
               gru_Wi, gru_bi, gru_Wh, gru_bh, Wp1, bp1, Wp2, bp2):
    import ml_dtypes
    f = np.float32
    E4 = ml_dtypes.float8_e4m3
    memory = np.ascontiguousarray(np.asarray(memory), dtype=f)
    od_mat = np.asarray(od_mat)
    walks = np.asarray(walks).astype(np.int64)
    W_rw = np.asarray(W_rw, dtype=f); b_rw = np.asarray(b_rw, dtype=f)
    W1 = np.asarray(W1, dtype=f); b1 = np.asarray(b1, dtype=f)
    W2 = np.asarray(W2, dtype=f); b2 = np.asarray(b2, dtype=f)
    gru_Wi = np.asarray(gru_Wi, dtype=f); gru_bi = np.asarray(gru_bi, dtype=f)
    gru_Wh = np.asarray(gru_Wh, dtype=f); gru_bh = np.asarray(gru_bh, dtype=f)
    Wp1 = np.asarray(Wp1, dtype=f); bp1 = np.asarray(bp1, dtype=f)
    Wp2 = np.asarray(Wp2, dtype=f); bp2 = np.asarray(bp2, dtype=f)

    W1dest = W1[:, 0:MD]
    W1od = W1[:, MD:MD + N]
    W1rw = W1[:, MD + N:]
    W1g = (W1rw @ W_rw) / np.float32(8.0)

    # od k-tiles: fp8(16 * W1od.T), k-subtile-major for DoubleRow pairs
    W1odT = np.zeros((32 * 128, HIDP), dtype=f)
    W1odT[:, :HID] = W1od.T * np.float32(SC)
    w1h = np.ascontiguousarray(
        W1odT.reshape(32, 128, HT, 128)
        .transpose(2, 1, 0, 3).reshape(HT, 128, 32 * 128).astype(E4))

    # mixed k-tile weights [dest | walk], x256 to match the fp8 product scale
    W1m = np.concatenate([W1dest, W1g], axis=1) * np.float32(SC * SC)  # [2112,128]
    W1mT = np.zeros((128, HIDP), dtype=f)
    W1mT[:, :HID] = W1m.T
    w1m = np.ascontiguousarray(W1mT.astype(ml_dtypes.bfloat16))

    # b1 fold: rw bias + the exact od mean-centering rank-1 term, x256
    b1p = np.zeros(HIDP, dtype=f)
    b1p[:HID] = (b1 + W1rw @ b_rw + np.float32(0.5) * W1od.sum(axis=1)) \
        * np.float32(SC * SC)

    # W2 absorbs the 1/256 descale
    W2tp = np.zeros((HIDP, MSG), dtype=f)
    W2tp[:HID] = W2.T / np.float32(SC * SC)
    w2t = np.ascontiguousarray(
        W2tp.reshape(HT, 128, MSG).transpose(1, 0, 2).reshape(128, HT * MSG))

    # prediction: fold bp2 into wp2 as a 65th contraction row
    wp2x = np.concatenate(
        [np.ascontiguousarray(Wp2.T), bp2[None, :]], axis=0)  # [65, N]

    # fp8 memory in k-subtile-major layout (lhsT of the gather matmuls)
    mt8 = np.ascontiguousarray(
        memory.reshape(32, 128, MD).transpose(1, 0, 2)
        .reshape(128, 32 * MD).astype(E4))

    def pad128(v):
        o = np.zeros(128, dtype=f)
        o[:v.shape[0]] = v
        return o

    # biases packed as [128 partitions, 64 columns]
    biases = np.zeros((64, 128), dtype=f)
    biases[0:HT] = b1p.reshape(HT, 128)
    biases[17] = gru_bi[:128] + gru_bh[:128]
    biases[18] = pad128(b2)
    biases[19] = pad128(gru_bi[128:])
    biases[20] = pad128(gru_bh[128:])
    biases[21] = pad128(bp1)
    biases = np.ascontiguousarray(biases.T)                    # [128, 64]

    shared = {
        "w1h": w1h,
        "w1m": w1m,
        "w2t": w2t,
        "mt8": mt8,
        "wi": np.ascontiguousarray(gru_Wi.T),
        "wh": np.ascontiguousarray(gru_Wh.T),
        "wp1": np.ascontiguousarray(Wp1.T),
        "wp2": np.ascontiguousarray(wp2x),
        "biases": biases,
        "ones": np.ones((1, R), dtype=f),
    }
    cols = np.tile(np.arange(R, dtype=np.int64)[:, None], (1, WL)).ravel()
    in_maps = []
    for c in range(NC):
        sl = slice(c * R, (c + 1) * R)
        odc = np.asarray(od_mat[sl], dtype=f)
        # odv[p, k*R+n] = 16*(od[c*R+n, k*128+p] - 0.5), fp8
        odv = np.ascontiguousarray(
            ((odc.T - np.float32(0.5)) * np.float32(SC))
            .reshape(32, 128, R).transpose(1, 0, 2).reshape(128, 32 * R)
            .astype(E4))
        # walk count matrix A[m, n] = #{j: walks[c*R+n, j] = m}, fp8 exact
        A = np.zeros((N, R), dtype=f)
        np.add.at(A, (walks[sl].ravel(), cols), np.float32(1.0))
        aw = np.ascontiguousarray(
            A.reshape(32, 128, R).transpose(1, 0, 2).reshape(128, 32 * R)
            .astype(E4))
        memT = np.ascontiguousarray(memory[sl].T)
        in_maps.append(dict(
            shared,
            memT=np.ascontiguousarray(memT.astype(ml_dtypes.bfloat16)),
            memf=memT,
            odv=odv,
            aw=aw,
        ))
    return in_maps


def _assemble(results):
    od = np.empty((N, N), dtype=np.float32)
    for c in range(NC):
        # outm[p, m*R+n] = od[c*R+n, m*128+p]
        od[c * R:(c + 1) * R, :] = (
            results[c]["outm"].astype(np.float32)
            .reshape(128, 32, R).transpose(2, 1, 0).reshape(R, N))
    return od


def _install_ntff_shim():
    """The agent image's antenv lacks axon_hooks, so trace=True dies on
    import. Recreate the module with the ctypes-based NTFF hook that
    trn_agent_boot would have registered."""
    import sys
    import types
    if "antenv.axon_hooks" in sys.modules:
        return
    from trn_agent_boot.trn_boot import _ntff_profile_via_ctypes
    hook = _ntff_profile_via_ctypes("/opt/axon/libaxon_pjrt.so")
    mod = types.ModuleType("antenv.axon_hooks")
    mod._hook = hook
    mod.get_axon_ntff_profile_hook = lambda: mod._hook
    mod.set_axon_ntff_profile_hook = lambda h: setattr(mod, "_hook", h)
    sys.modules["antenv.axon_hooks"] = mod


def run(inputs, trace=False):
    """Run on 8 NeuronCores; returns (od [N,N] f32, BassKernelResults)."""
    from concourse.bass_utils import run_bass_kernel_spmd
    if trace:
        try:
            _install_ntff_shim()
        except Exception as e:
            print(f"ntff shim failed ({e}); running without trace")
            trace = False
    nc = _get_program()
    in_maps = _host_prep(**inputs)
    res = run_bass_kernel_spmd(nc, in_maps, list(range(NC)), trace=trace)
    return _assemble(res.results), res


def kernel(**inputs):
    od, _ = run(inputs)
    return od


# revision 23
# speedup vs baseline: 1.4066x; 1.0206x over previous
"""Trainium2 Bass kernel for nn_DiscreteModel (GNN message passing).

Strategy: shard by node rows across 8 cores (512 rows each). All per-node
tensors are kept feature-major ([feature, node]) on-chip so the contraction
dim of every matmul sits on SBUF partitions. The host pre-transposes the
od_mat shard and all weights, folds the random-walk projection W_rw and the
1/8 mean into the layer-1 weight block, and pads HID 2112 -> 2176.

The dominant cost is the od x W1 layer-1 stream (4096 of the 4224 k dims).
It runs in fp8 e4m3 DoubleRow mode (2 k-subtiles per matmul, 2x PE rate):
od is mean-centered (od = 0.5 + odc, the exact rank-1 term 0.5*rowsum(W1od)
folds into b1) so fp8 quantization noise halves relative to signal, both
operands are scaled x16 and the x256 product scale rides through the relu
(relu(256x) = 256*relu(x)) and divides out of W2. Host-side fp8 rel err on
the final output: 5.8e-3 (gate 2e-2).

The walk gather is a matmul: the host one-hot-encodes walks into the count
matrix A[m, n] = #{j: walks[n, j] = m} (exact small ints in fp8), and
GsT = sum_k mem_k.T @ A_k runs as 16 DoubleRow matmuls (~3.5us) as soon as
A lands. This replaces 32 indirect DMAs whose descriptor generation
serialized ~35us on the gpsimd engine (hw reads exactly ONE offset per
partition per indirect DMA instruction, so they cannot be merged).

Pipeline per core (n = 512 node columns):
  gather  : GsT = mem.T @ A via DoubleRow fp8 -> bottom half of the "mixed"
            k-tile (top half = memT shard, DMA'd directly). Mixed is bf16.
  layer1  : hiddenT[h] = relu(sum_k W1pT[k,h].T @ rawT[k] + b1), k = 16
            DoubleRow od pairs (resident, 2MB fp8) + 1 mixed tile; 17
            h-tiles of 128.
  layer2  : msgT accumulates W2T[h].T @ hiddenT[h] in one PSUM bank.
  GRU     : gate matmuls K=64 at half width, gates via ACT sigmoid/tanh +
            DVE/gpsimd ops in quarter-width chunks for pipeline depth.
  predict : actT = relu(Wp1 @ updT + bp1); outT[m] = Wp2xT[m].T @ actxT
            with bp2 folded in as a 65th contraction row against ones.
Non-fp8 matmuls run in float32r (1 cycle/row at free dim >= 256). Output
is written bf16 (4 m-tiles per DMA) and upcast on the host.
"""

import numpy as np

import concourse.bass as bass
import concourse.bacc as bacc
import concourse.tile as tile
from concourse import mybir

N = 4096        # nodes
MD = 64         # memory dim
MSG = 64        # message dim
WL = 8          # walk length
HID = 2112
HT = 17         # h-tiles (HID padded to 17*128 = 2176)
HIDP = HT * 128
NC = 8          # cores
R = N // NC     # rows (nodes) per core = 512
NT = R // 128   # node tiles per core = 4
F32 = mybir.dt.float32
F32R = mybir.dt.float32r
BF16 = mybir.dt.bfloat16
FP8 = mybir.dt.float8e4
I32 = mybir.dt.int32

SC = 16.0       # per-operand fp8 scale; product scale 256 rides to W2
DELAY = 6       # h-groups the mixed k-tile matmul trails by
_PROG = None


def _build_program():
    nc = bacc.Bacc("TRN2", target_bir_lowering=False, debug=False, num_devices=NC)

    # ---- DRAM I/O (all pre-laid-out on host, partition-major) ----
    memT_d = nc.dram_tensor("memT", [MD, R], BF16, kind="ExternalInput").ap()
    memf_d = nc.dram_tensor("memf", [MD, R], F32R, kind="ExternalInput").ap()
    od_d = nc.dram_tensor("odv", [128, 32 * R], FP8, kind="ExternalInput").ap()
    aw_d = nc.dram_tensor("aw", [128, 32 * R], FP8, kind="ExternalInput").ap()
    mt8_d = nc.dram_tensor("mt8", [128, 32 * MD], FP8, kind="ExternalInput").ap()
    w1h_d = nc.dram_tensor("w1h", [HT, 128, 32 * 128], FP8, kind="ExternalInput").ap()
    w1m_d = nc.dram_tensor("w1m", [128, HT * 128], BF16, kind="ExternalInput").ap()
    w2t_d = nc.dram_tensor("w2t", [128, HT * MSG], F32R, kind="ExternalInput").ap()
    wi_d = nc.dram_tensor("wi", [MSG, 3 * MD], F32R, kind="ExternalInput").ap()
    wh_d = nc.dram_tensor("wh", [MD, 3 * MD], F32R, kind="ExternalInput").ap()
    wp1_d = nc.dram_tensor("wp1", [MD, MD], F32R, kind="ExternalInput").ap()
    wp2_d = nc.dram_tensor("wp2", [MD + 1, N], F32R, kind="ExternalInput").ap()
    bias_d = nc.dram_tensor("biases", [128, 64], F32, kind="ExternalInput").ap()
    ones_d = nc.dram_tensor("ones", [1, R], F32R, kind="ExternalInput").ap()
    out_d = nc.dram_tensor("outm", [128, 32 * R], BF16, kind="ExternalOutput").ap()

    AF = mybir.ActivationFunctionType
    DR = mybir.MatmulPerfMode.DoubleRow

    with tile.TileContext(nc) as tc:
        with (
            tc.tile_pool(name="consts", bufs=1) as consts,
            tc.tile_pool(name="w1p", bufs=3) as w1p,
            tc.tile_pool(name="hp", bufs=3) as hp,
            tc.tile_pool(name="gates", bufs=1) as gates,
            tc.tile_pool(name="ostg", bufs=3) as ostg,
            tc.tile_pool(name="pmm", bufs=6, space="PSUM") as pmm,
            tc.tile_pool(name="pacc", bufs=1, space="PSUM") as pacc,
        ):
            # ---- od shard first: the h=0 matmul march is paced by od
            #      arrival. Split the chunks across the sync and gpsimd
            #      queues (DMA issue is only possible from sync/scalar/
            #      gpsimd); the w1h h=0 tile leads the sync queue. ----
            odres = consts.tile([128, 32 * R], FP8, tag="odres")
            nc.sync.dma_start(out=odres[:, 0:2 * R], in_=od_d[:, 0:2 * R])
            w1t0 = w1p.tile([128, 32 * 128], FP8, tag="w1t")
            HKC = 16 * 128
            nc.sync.dma_start(out=w1t0[:, :HKC], in_=w1h_d[0][:, :HKC])
            nc.sync.dma_start(out=w1t0[:, HKC:], in_=w1h_d[0][:, HKC:])
            nc.sync.dma_start(out=odres[:, 2 * R:8 * R], in_=od_d[:, 2 * R:8 * R])
            for c0, cn in [(8, 8), (16, 8), (24, 8)]:
                nc.gpsimd.dma_start(
                    out=odres[:, c0 * R:(c0 + cn) * R],
                    in_=od_d[:, c0 * R:(c0 + cn) * R],
                )

            # walk count matrix + fp8 memory for the matmul gather; these
            # queue behind every od chunk so they never starve the h=0 march
            mt8 = consts.tile([128, 32 * MD], FP8, tag="mt8")
            nc.gpsimd.dma_start(out=mt8[:], in_=mt8_d[:])
            aw = consts.tile([128, 32 * R], FP8, tag="aw")
            for c in range(4):
                nc.gpsimd.dma_start(
                    out=aw[:, c * 8 * R:(c + 1) * 8 * R],
                    in_=aw_d[:, c * 8 * R:(c + 1) * 8 * R])

            biasp = consts.tile([128, 64], F32, tag="biasp")
            nc.scalar.dma_start(out=biasp[:], in_=bias_d[:])

            # mixed rawT k-tile: [0:64] = memT shard, [64:128] = GsT (walk sums)
            mixed = consts.tile([128, R], BF16, tag="mixed")
            nc.scalar.dma_start(out=mixed[0:MD, :], in_=memT_d[:])
            memf = consts.tile([MD, R], F32R, tag="memf")
            nc.scalar.dma_start(out=memf[:], in_=memf_d[:])

            mixed_r = mixed[:]
            odres3 = odres[:].rearrange("p (k n) -> p k n", k=32)
            aw3 = aw[:].rearrange("p (k n) -> p k n", k=32)
            mt3 = mt8[:].rearrange("p (k d) -> p k d", k=32)

            # ---- layer 1 + layer 2 accumulation ----
            ps_msg = pacc.tile([MSG, R], F32, tag="msg")
            pending = []

            def finalize(h, ps):
                # mixed k-tile contribution arrives late (gather matmuls);
                # delaying it by DELAY h-groups keeps the PE dense.
                nc.tensor.matmul(
                    out=ps[:],
                    lhsT=w1m_sb[:, h * 128:(h + 1) * 128],
                    rhs=mixed_r,
                    start=False, stop=True,
                )
                hid = hp.tile([128, R], F32R, tag="hid")
                nc.scalar.activation(hid[:], ps[:], AF.Relu, bias=biasp[:, h:h + 1])
                nc.tensor.matmul(
                    out=ps_msg[:],
                    lhsT=w2t_sb[:, h * MSG:(h + 1) * MSG],
                    rhs=hid[:],
                    start=(h == 0), stop=(h == HT - 1),
                )

            for h in range(HT):
                if h == 2:
                    w1m_sb = consts.tile([128, HT * 128], BF16, tag="w1m")
                    nc.scalar.dma_start(out=w1m_sb[:], in_=w1m_d[:])
                    w2t_sb = consts.tile([128, HT * MSG], F32R, tag="w2t")
                    nc.scalar.dma_start(out=w2t_sb[:], in_=w2t_d[:])
                    wh_sb = consts.tile([MD, 3 * MD], F32R, tag="wh")
                    nc.scalar.dma_start(out=wh_sb[:], in_=wh_d[:])
                if h == 3:
                    # h_n = memT @ Wh_n and its bias add depend only on memf;
                    # do them in the DMA-paced ramp where the PE has slack
                    ps_hn = pmm.tile([MD, R], F32, tag="mm")
                    nc.tensor.matmul(out=ps_hn[:], lhsT=wh_sb[:, 128:192],
                                     rhs=memf[:], start=True, stop=True)
                    hnb = gates.tile([MD, R], F32, tag="hnb")
                    nc.vector.tensor_scalar_add(out=hnb[:], in0=ps_hn[:],
                                                scalar1=biasp[0:MD, 20:21])
                if h == 4:
                    # walk gather on the PE: GsT = sum_k mem_k.T @ A_k
                    gsps = pacc.tile([MD, R], F32, tag="pred")
                    for kk in range(16):
                        nc.tensor.matmul(
                            out=gsps[:],
                            lhsT=mt3[:, 2 * kk:2 * kk + 2, :],
                            rhs=aw3[:, 2 * kk:2 * kk + 2, :],
                            start=(kk == 0), stop=(kk == 15),
                            perf_mode=DR,
                        )
                    nc.vector.tensor_copy(out=mixed[MD:128, :], in_=gsps[:])
                if h == 0:
                    w1t = w1t0
                else:
                    w1t = w1p.tile([128, 32 * 128], FP8, tag="w1t")
                    eng = nc.scalar if h % 4 == 3 else nc.sync
                    eng.dma_start(out=w1t[:], in_=w1h_d[h])
                w1t3 = w1t[:].rearrange("p (k c) -> p k c", k=32)
                ps = pmm.tile([128, R], F32, tag="mm")
                for kk in range(16):
                    nc.tensor.matmul(
                        out=ps[:],
                        lhsT=w1t3[:, 2 * kk:2 * kk + 2, :],
                        rhs=odres3[:, 2 * kk:2 * kk + 2, :],
                        start=(kk == 0), stop=False,
                        perf_mode=DR,
                    )
                pending.append((h, ps))
                if len(pending) > DELAY:
                    finalize(*pending.pop(0))
                if h >= 12 and pending:
                    # the gather is long done by now; drain eagerly so the
                    # tail does not bunch 5 finalizes after the last group
                    finalize(*pending.pop(0))
            for h, ps in pending:
                finalize(h, ps)

            wi_sb = consts.tile([MSG, 3 * MD], F32R, tag="wi")
            nc.scalar.dma_start(out=wi_sb[:], in_=wi_d[:])
            wp1_sb = consts.tile([MD, MD], F32R, tag="wp1")
            nc.scalar.dma_start(out=wp1_sb[:], in_=wp1_d[:])
            wp2_sb = consts.tile([MD + 1, N], F32R, tag="wp2")
            nc.scalar.dma_start(out=wp2_sb[:], in_=wp2_d[:])

            msg_sb = gates.tile([MSG, R], F32R, tag="msg_sb")
            nc.vector.tensor_scalar_add(
                out=msg_sb[:], in0=ps_msg[:], scalar1=biasp[0:MSG, 18:19])
            msg_r = msg_sb[:]
            memT_r = memf[:]

            # ---- GRU + prediction. Matmuls run at half width (f32r needs
            #      free >= 256); the serial ACT/DVE/gpsimd gate chain runs in
            #      quarter-width chunks so it pipelines deeper. ----
            ps_rz = pmm.tile([128, R], F32, tag="mm")
            nc.tensor.matmul(out=ps_rz[:], lhsT=wi_sb[:, 0:128], rhs=msg_r,
                             start=True, stop=False)
            nc.tensor.matmul(out=ps_rz[:], lhsT=wh_sb[:, 0:128], rhs=memT_r,
                             start=False, stop=True)
            ps_in = pmm.tile([MD, R], F32, tag="mm")
            nc.tensor.matmul(out=ps_in[:], lhsT=wi_sb[:, 128:192], rhs=msg_r,
                             start=True, stop=True)
            rz = gates.tile([128, R], F32, tag="rz")
            rhn = gates.tile([MD, R], F32, tag="rhn")
            npre = gates.tile([MD, R], F32, tag="npre")
            n_t = gates.tile([MD, R], F32, tag="n_t")
            d_t = gates.tile([MD, R], F32, tag="d_t")
            z_t = gates.tile([MD, R], F32, tag="z_t")
            zd = gates.tile([MD, R], F32, tag="zd")
            # actx rows 0:64 = relu(pred1); row 64 = ones so the out matmul's
            # 65th contraction row adds bp2 (folded into wp2 row 64)
            actx = gates.tile([MD + 1, R], F32R, tag="actx")
            nc.scalar.dma_start(out=actx[MD:MD + 1, :], in_=ones_d[:])
            upd = gates.tile([MD, R], F32R, tag="upd")
            ps_pred = pacc.tile([MD, R], F32, tag="pred")
            QR = R // 4
            for x in range(4):
                cs = slice(x * QR, (x + 1) * QR)
                nc.scalar.activation(rz[:, cs], ps_rz[:, cs], AF.Sigmoid,
                                     bias=biasp[:, 17:18])
                nc.vector.tensor_mul(out=rhn[:, cs], in0=rz[0:MD, cs], in1=hnb[:, cs])
                nc.vector.tensor_add(out=npre[:, cs], in0=ps_in[:, cs], in1=rhn[:, cs])
                nc.scalar.activation(n_t[:, cs], npre[:, cs], AF.Tanh,
                                     bias=biasp[0:MD, 19:20])
                nc.gpsimd.tensor_sub(out=d_t[:, cs], in0=memf[:, cs].bitcast(F32),
                                      in1=n_t[:, cs])
                nc.gpsimd.tensor_copy(out=z_t[:, cs], in_=rz[MD:128, cs])
                nc.vector.tensor_mul(out=zd[:, cs], in0=z_t[:, cs], in1=d_t[:, cs])
                nc.vector.tensor_add(out=upd[:, cs], in0=n_t[:, cs], in1=zd[:, cs])
                if x % 2 == 1:
                    hs = slice((x - 1) * QR, (x + 1) * QR)
                    nc.tensor.matmul(out=ps_pred[:, hs], lhsT=wp1_sb[:],
                                     rhs=upd[:, hs], start=True, stop=True)
                    nc.scalar.activation(actx[0:MD, hs], ps_pred[:, hs], AF.Relu,
                                         bias=biasp[0:MD, 21:22])
            act_r = actx[:]

            for m in range(32):
                ps_o = pmm.tile([128, R], F32, tag="mm")
                nc.tensor.matmul(out=ps_o[:], lhsT=wp2_sb[:, m * 128:(m + 1) * 128],
                                 rhs=act_r, start=True, stop=True)
                if m % 4 == 0:
                    stage = ostg.tile([128, 4 * R], BF16, tag="stage")
                dst = stage[:, (m % 4) * R:(m % 4 + 1) * R]
                # alternate PSUM->SBUF bf16 casts across DVE and ACT (gpsimd
                # cannot read PSUM) so neither engine serializes the tail
                if m % 2 == 0:
                    nc.vector.tensor_copy(out=dst, in_=ps_o[:])
                else:
                    nc.scalar.activation(dst, ps_o[:], AF.Copy)
                if m % 4 == 3:
                    nc.sync.dma_start(
                        out=out_d[:, (m - 3) * R:(m + 1) * R], in_=stage[:])

    nc.compile()
    return nc


def _get_program():
    global _PROG
    if _PROG is None:
        _PROG = _build_program()
    return _PROG


def _host_prep(memory, od_mat, walks, W_rw, b_rw, W1, b1, W2, b2,
               gru_Wi, gru_bi, gru_Wh, gru_bh, Wp1, bp1, Wp2, bp2):
    import ml_dtypes
    f = np.float32
    E4 = ml_dtypes.float8_e4m3
    memory = np.ascontiguousarray(np.asarray(memory), dtype=f)
    od_mat = np.asarray(od_mat)
    walks = np.asarray(walks).astype(np.int64)
    W_rw = np.asarray(W_rw, dtype=f); b_rw = np.asarray(b_rw, dtype=f)
    W1 = np.asarray(W1, dtype=f); b1 = np.asarray(b1, dtype=f)
    W2 = np.asarray(W2, dtype=f); b2 = np.asarray(b2, dtype=f)
    gru_Wi = np.asarray(gru_Wi, dtype=f); gru_bi = np.asarray(gru_bi, dtype=f)
    gru_Wh = np.asarray(gru_Wh, dtype=f); gru_bh = np.asarray(gru_bh, dtype=f)
    Wp1 = np.asarray(Wp1, dtype=f); bp1 = np.asarray(bp1, dtype=f)
    Wp2 = np.asarray(Wp2, dtype=f); bp2 = np.asarray(bp2, dtype=f)

    W1dest = W1[:, 0:MD]
    W1od = W1[:, MD:MD + N]
    W1rw = W1[:, MD + N:]
    W1g = (W1rw @ W_rw) / np.float32(8.0)

    # od k-tiles: fp8(16 * W1od.T), k-subtile-major for DoubleRow pairs
    W1odT = np.zeros((32 * 128, HIDP), dtype=f)
    W1odT[:, :HID] = W1od.T * np.float32(SC)
    w1h = np.ascontiguousarray(
        W1odT.reshape(32, 128, HT, 128)
        .transpose(2, 1, 0, 3).reshape(HT, 128, 32 * 128).astype(E4))

    # mixed k-tile weights [dest | walk], x256 to match the fp8 product scale
    W1m = np.concatenate([W1dest, W1g], axis=1) * np.float32(SC * SC)  # [2112,128]
    W1mT = np.zeros((128, HIDP), dtype=f)
    W1mT[:, :HID] = W1m.T
    w1m = np.ascontiguousarray(W1mT.astype(ml_dtypes.bfloat16))

    # b1 fold: rw bias + the exact od mean-centering rank-1 term, x256
    b1p = np.zeros(HIDP, dtype=f)
    b1p[:HID] = (b1 + W1rw @ b_rw + np.float32(0.5) * W1od.sum(axis=1)) \
        * np.float32(SC * SC)

    # W2 absorbs the 1/256 descale
    W2tp = np.zeros((HIDP, MSG), dtype=f)
    W2tp[:HID] = W2.T / np.float32(SC * SC)
    w2t = np.ascontiguousarray(
        W2tp.reshape(HT, 128, MSG).transpose(1, 0, 2).reshape(128, HT * MSG))

    # prediction: fold bp2 into wp2 as a 65th contraction row
    wp2x = np.concatenate(
        [np.ascontiguousarray(Wp2.T), bp2[None, :]], axis=0)  # [65, N]

    # fp8 memory in k-subtile-major layout (lhsT of the gather matmuls)
    mt8 = np.ascontiguousarray(
        memory.reshape(32, 128, MD).transpose(1, 0, 2)
        .reshape(128, 32 * MD).astype(E4))

    def pad128(v):
        o = np.zeros(128, dtype=f)
        o[:v.shape[0]] = v
        return o

    # biases packed as [128 partitions, 64 columns]
    biases = np.zeros((64, 128), dtype=f)
    biases[0:HT] = b1p.reshape(HT, 128)
    biases[17] = gru_bi[:128] + gru_bh[:128]
    biases[18] = pad128(b2)
    biases[19] = pad128(gru_bi[128:])
    biases[20] = pad128(gru_bh[128:])
    biases[21] = pad128(bp1)
    biases = np.ascontiguousarray(biases.T)                    # [128, 64]

    shared = {
        "w1h": w1h,
        "w1m": w1m,
        "w2t": w2t,
        "mt8": mt8,
        "wi": np.ascontiguousarray(gru_Wi.T),
        "wh": np.ascontiguousarray(gru_Wh.T),
        "wp1": np.ascontiguousarray(Wp1.T),
        "wp2": np.ascontiguousarray(wp2x),
        "biases": biases,
        "ones": np.ones((1, R), dtype=f),
    }
    cols = np.tile(np.arange(R, dtype=np.int64)[:, None], (1, WL)).ravel()
    in_maps = []
    for c in range(NC):
        sl = slice(c * R, (c + 1) * R)
        odc = np.asarray(od_mat[sl], dtype=f)
        # odv[p, k*R+n] = 16*(od[c*R+n, k*128+p] - 0.5), fp8
        odv = np.ascontiguousarray(
            ((odc.T - np.float32(0.5)) * np.float32(SC))
            .reshape(32, 128, R).transpose(1, 0, 2).reshape(128, 32 * R)
            .astype(E4))
        # walk count matrix A[m, n] = #{j: walks[c*R+n, j] = m}, fp8 exact
        A = np.zeros((N, R), dtype=f)
        np.add.at(A, (walks[sl].ravel(), cols), np.float32(1.0))
        aw = np.ascontiguousarray(
            A.reshape(32, 128, R).transpose(1, 0, 2).reshape(128, 32 * R)
            .astype(E4))
        memT = np.ascontiguousarray(memory[sl].T)
        in_maps.append(dict(
            shared,
            memT=np.ascontiguousarray(memT.astype(ml_dtypes.bfloat16)),
            memf=memT,
            odv=odv,
            aw=aw,
        ))
    return in_maps


def _assemble(results):
    od = np.empty((N, N), dtype=np.float32)
    for c in range(NC):
        # outm[p, m*R+n] = od[c*R+n, m*128+p]
        od[c * R:(c + 1) * R, :] = (
            results[c]["outm"].astype(np.float32)
            .reshape(128, 32, R).transpose(2, 1, 0).reshape(R, N))
    return od


def _install_ntff_shim():
    """The agent image's antenv lacks axon_hooks, so trace=True dies on
    import. Recreate the module with the ctypes-based NTFF hook that
    trn_agent_boot would have registered."""
    import sys
    import types
    if "antenv.axon_hooks" in sys.modules:
        return
    from trn_agent_boot.trn_boot import _ntff_profile_via_ctypes
    hook = _ntff_profile_via_ctypes("/opt/axon/libaxon_pjrt.so")
    mod = types.ModuleType("antenv.axon_hooks")
    mod._hook = hook
    mod.get_axon_ntff_profile_hook = lambda: mod._hook
    mod.set_axon_ntff_profile_hook = lambda h: setattr(mod, "_hook", h)
    sys.modules["antenv.axon_hooks"] = mod


def run(inputs, trace=False):
    """Run on 8 NeuronCores; returns (od [N,N] f32, BassKernelResults)."""
    from concourse.bass_utils import run_bass_kernel_spmd
    if trace:
        try:
            _install_ntff_shim()
        except Exception as e:
            print(f"ntff shim failed ({e}); running without trace")
            trace = False
    nc = _get_program()
    in_maps = _host_prep(**inputs)
    res = run_bass_kernel_spmd(nc, in_maps, list(range(NC)), trace=trace)
    return _assemble(res.results), res


def kernel(**inputs):
    od, _ = run(inputs)
    return od


# revision 26
# speedup vs baseline: 1.4604x; 1.0382x over previous
"""Trainium2 Bass kernel for nn_DiscreteModel (GNN message passing).

Strategy: shard by node rows across 8 cores (512 rows each). All per-node
tensors are kept feature-major ([feature, node]) on-chip so the contraction
dim of every matmul sits on SBUF partitions. The host pre-transposes the
od_mat shard and all weights, folds the random-walk projection W_rw and the
1/8 mean into the layer-1 weight block, and pads HID 2112 -> 2176.

The dominant cost is the od x W1 layer-1 stream (4096 of the 4224 k dims).
It runs in fp8 e4m3 DoubleRow mode (2 k-subtiles per matmul, 2x PE rate):
od is mean-centered (od = 0.5 + odc, the exact rank-1 term 0.5*rowsum(W1od)
folds into b1) so fp8 quantization noise halves relative to signal, both
operands are scaled x16 and the x256 product scale rides through the relu
(relu(256x) = 256*relu(x)) and divides out of W2. Host-side fp8 rel err on
the final output: 5.8e-3 (gate 2e-2).

The walk gather is a matmul: the host one-hot-encodes walks into the count
matrix A[m, n] = #{j: walks[n, j] = m} (exact small ints in fp8), and
GsT = sum_k mem_k.T @ A_k runs as 16 DoubleRow matmuls (~3.5us) as soon as
A lands. This replaces 32 indirect DMAs whose descriptor generation
serialized ~35us on the gpsimd engine (hw reads exactly ONE offset per
partition per indirect DMA instruction, so they cannot be merged).

Pipeline per core (n = 512 node columns):
  gather  : GsT = mem.T @ A via DoubleRow fp8 -> bottom half of the "mixed"
            k-tile (top half = memT shard, DMA'd directly). Mixed is bf16.
  layer1  : hiddenT[h] = relu(sum_k W1pT[k,h].T @ rawT[k] + b1), k = 16
            DoubleRow od pairs (resident, 2MB fp8) + 1 mixed tile; 17
            h-tiles of 128.
  layer2  : msgT accumulates W2T[h].T @ hiddenT[h] in one PSUM bank.
  GRU     : gate matmuls K=64 at half width, gates via ACT sigmoid/tanh +
            DVE/gpsimd ops in quarter-width chunks for pipeline depth.
  predict : actT = relu(Wp1 @ updT + bp1); outT[m] = Wp2xT[m].T @ actxT
            with bp2 folded in as a 65th contraction row against ones.
Non-fp8 matmuls run in float32r (1 cycle/row at free dim >= 256). Output
is written bf16 (4 m-tiles per DMA) and upcast on the host.
"""

import numpy as np

import concourse.bass as bass
import concourse.bacc as bacc
import concourse.tile as tile
from concourse import mybir

N = 4096        # nodes
MD = 64         # memory dim
MSG = 64        # message dim
WL = 8          # walk length
HID = 2112
HT = 17         # h-tiles (HID padded to 17*128 = 2176)
HIDP = HT * 128
NC = 8          # cores
R = N // NC     # rows (nodes) per core = 512
NT = R // 128   # node tiles per core = 4
F32 = mybir.dt.float32
F32R = mybir.dt.float32r
BF16 = mybir.dt.bfloat16
FP8 = mybir.dt.float8e4
I32 = mybir.dt.int32

SC = 16.0       # per-operand fp8 scale; product scale 256 rides to W2
DELAY = 6       # h-groups the mixed k-tile matmul trails by
_PROG = None


def _build_program():
    nc = bacc.Bacc("TRN2", target_bir_lowering=False, debug=False, num_devices=NC)

    # ---- DRAM I/O (all pre-laid-out on host, partition-major) ----
    memT_d = nc.dram_tensor("memT", [MD, R], BF16, kind="ExternalInput").ap()
    memf_d = nc.dram_tensor("memf", [MD, R], F32R, kind="ExternalInput").ap()
    od_d = nc.dram_tensor("odv", [128, 32 * R], FP8, kind="ExternalInput").ap()
    aw_d = nc.dram_tensor("aw", [128, 32 * R], FP8, kind="ExternalInput").ap()
    mt8_d = nc.dram_tensor("mt8", [128, 32 * MD], FP8, kind="ExternalInput").ap()
    w1h_d = nc.dram_tensor("w1h", [HT, 128, 32 * 128], FP8, kind="ExternalInput").ap()
    w1m_d = nc.dram_tensor("w1m", [128, HT * 128], BF16, kind="ExternalInput").ap()
    w2t_d = nc.dram_tensor("w2t", [128, HT * MSG], F32R, kind="ExternalInput").ap()
    wi_d = nc.dram_tensor("wi", [MSG, 3 * MD], F32R, kind="ExternalInput").ap()
    wh_d = nc.dram_tensor("wh", [MD, 3 * MD], F32R, kind="ExternalInput").ap()
    wp1_d = nc.dram_tensor("wp1", [MD, MD], F32R, kind="ExternalInput").ap()
    wp2_d = nc.dram_tensor("wp2", [MD + 1, N], F32R, kind="ExternalInput").ap()
    bias_d = nc.dram_tensor("biases", [128, 64], F32, kind="ExternalInput").ap()
    ones_d = nc.dram_tensor("ones", [1, R], F32R, kind="ExternalInput").ap()
    out_d = nc.dram_tensor("outm", [128, 32 * R], BF16, kind="ExternalOutput").ap()

    AF = mybir.ActivationFunctionType
    DR = mybir.MatmulPerfMode.DoubleRow

    with tile.TileContext(nc) as tc:
        with (
            tc.tile_pool(name="consts", bufs=1) as consts,
            tc.tile_pool(name="w1p", bufs=3) as w1p,
            tc.tile_pool(name="hp", bufs=3) as hp,
            tc.tile_pool(name="gates", bufs=1) as gates,
            tc.tile_pool(name="ostg", bufs=3) as ostg,
            tc.tile_pool(name="pmm", bufs=6, space="PSUM") as pmm,
            tc.tile_pool(name="pacc", bufs=1, space="PSUM") as pacc,
        ):
            # ---- od shard first: the h=0 matmul march is paced by od
            #      arrival. Split the chunks across the sync and gpsimd
            #      queues (DMA issue is only possible from sync/scalar/
            #      gpsimd); the w1h h=0 tile leads the sync queue. ----
            # Each DMA queue sustains only ~110GB/s, so the od shard (the
            # tensor pacing the h=0..2 march) is cut into 2-ktile chunks
            # round-robined over all three queues; w1h0 leads the sync queue.
            odres = consts.tile([128, 32 * R], FP8, tag="odres")
            w1t0 = w1p.tile([128, 32 * 128], FP8, tag="w1t")
            HKC = 16 * 128
            nc.sync.dma_start(out=w1t0[:, :HKC], in_=w1h_d[0][:, :HKC])
            nc.sync.dma_start(out=w1t0[:, HKC:], in_=w1h_d[0][:, HKC:])
            QE = [nc.sync, nc.gpsimd, nc.scalar]
            for i in range(16):
                QE[i % 3].dma_start(
                    out=odres[:, i * 2 * R:(i + 1) * 2 * R],
                    in_=od_d[:, i * 2 * R:(i + 1) * 2 * R],
                )

            memf = consts.tile([MD, R], F32R, tag="memf")
            nc.scalar.dma_start(out=memf[:], in_=memf_d[:])
            biasp = consts.tile([128, 64], F32, tag="biasp")
            nc.scalar.dma_start(out=biasp[:], in_=bias_d[:])
            # mixed rawT k-tile: [0:64] = memT shard, [64:128] = GsT (walk sums)
            mixed = consts.tile([128, R], BF16, tag="mixed")
            nc.scalar.dma_start(out=mixed[0:MD, :], in_=memT_d[:])
            wh_sb = consts.tile([MD, 3 * MD], F32R, tag="wh")
            nc.scalar.dma_start(out=wh_sb[:], in_=wh_d[:])

            # walk count matrix + fp8 memory for the matmul gather; these
            # queue behind every od chunk so they never starve the h=0 march
            mt8 = consts.tile([128, 32 * MD], FP8, tag="mt8")
            nc.gpsimd.dma_start(out=mt8[:], in_=mt8_d[:])
            aw = consts.tile([128, 32 * R], FP8, tag="aw")
            for c, eng in enumerate([nc.scalar, nc.scalar, nc.gpsimd, nc.gpsimd]):
                eng.dma_start(
                    out=aw[:, c * 8 * R:(c + 1) * 8 * R],
                    in_=aw_d[:, c * 8 * R:(c + 1) * 8 * R])

            mixed_r = mixed[:]
            odres3 = odres[:].rearrange("p (k n) -> p k n", k=32)
            aw3 = aw[:].rearrange("p (k n) -> p k n", k=32)
            mt3 = mt8[:].rearrange("p (k d) -> p k d", k=32)

            # ---- layer 1 + layer 2 accumulation ----
            ps_msg = pacc.tile([MSG, R], F32, tag="msg")
            pending = []

            def finalize(h, ps):
                # mixed k-tile contribution arrives late (gather matmuls);
                # delaying it by DELAY h-groups keeps the PE dense.
                nc.tensor.matmul(
                    out=ps[:],
                    lhsT=w1m_sb[:, h * 128:(h + 1) * 128],
                    rhs=mixed_r,
                    start=False, stop=True,
                )
                hid = hp.tile([128, R], F32R, tag="hid")
                nc.scalar.activation(hid[:], ps[:], AF.Relu, bias=biasp[:, h:h + 1])
                nc.tensor.matmul(
                    out=ps_msg[:],
                    lhsT=w2t_sb[:, h * MSG:(h + 1) * MSG],
                    rhs=hid[:],
                    start=(h == 0), stop=(h == HT - 1),
                )

            for h in range(HT):
                if h == 2:
                    w1m_sb = consts.tile([128, HT * 128], BF16, tag="w1m")
                    nc.scalar.dma_start(out=w1m_sb[:], in_=w1m_d[:])
                    w2t_sb = consts.tile([128, HT * MSG], F32R, tag="w2t")
                    nc.scalar.dma_start(out=w2t_sb[:], in_=w2t_d[:])
                if h == 3:
                    # h_n = memT @ Wh_n and its bias add depend only on memf;
                    # do them in the DMA-paced ramp where the PE has slack
                    ps_hn = pmm.tile([MD, R], F32, tag="mm")
                    nc.tensor.matmul(out=ps_hn[:], lhsT=wh_sb[:, 128:192],
                                     rhs=memf[:], start=True, stop=True)
                    hnb = gates.tile([MD, R], F32, tag="hnb")
                    nc.vector.tensor_scalar_add(out=hnb[:], in0=ps_hn[:],
                                                scalar1=biasp[0:MD, 20:21])
                if h == 4:
                    # walk gather on the PE: GsT = sum_k mem_k.T @ A_k
                    gsps = pacc.tile([MD, R], F32, tag="pred")
                    for kk in range(16):
                        nc.tensor.matmul(
                            out=gsps[:],
                            lhsT=mt3[:, 2 * kk:2 * kk + 2, :],
                            rhs=aw3[:, 2 * kk:2 * kk + 2, :],
                            start=(kk == 0), stop=(kk == 15),
                            perf_mode=DR,
                        )
                    nc.vector.tensor_copy(out=mixed[MD:128, :], in_=gsps[:])
                if h == 0:
                    w1t = w1t0
                else:
                    w1t = w1p.tile([128, 32 * 128], FP8, tag="w1t")
                    nc.sync.dma_start(out=w1t[:], in_=w1h_d[h])
                w1t3 = w1t[:].rearrange("p (k c) -> p k c", k=32)
                ps = pmm.tile([128, R], F32, tag="mm")
                for kk in range(16):
                    nc.tensor.matmul(
                        out=ps[:],
                        lhsT=w1t3[:, 2 * kk:2 * kk + 2, :],
                        rhs=odres3[:, 2 * kk:2 * kk + 2, :],
                        start=(kk == 0), stop=False,
                        perf_mode=DR,
                    )
                pending.append((h, ps))
                if len(pending) > DELAY:
                    finalize(*pending.pop(0))
                if h >= 12 and pending:
                    # the gather is long done by now; drain eagerly so the
                    # tail does not bunch 5 finalizes after the last group
                    finalize(*pending.pop(0))
            for h, ps in pending:
                finalize(h, ps)

            wi_sb = consts.tile([MSG, 3 * MD], F32R, tag="wi")
            nc.scalar.dma_start(out=wi_sb[:], in_=wi_d[:])
            wp1_sb = consts.tile([MD, MD], F32R, tag="wp1")
            nc.scalar.dma_start(out=wp1_sb[:], in_=wp1_d[:])
            wp2_sb = consts.tile([MD + 1, N], F32R, tag="wp2")
            nc.scalar.dma_start(out=wp2_sb[:], in_=wp2_d[:])

            msg_sb = gates.tile([MSG, R], F32R, tag="msg_sb")
            nc.vector.tensor_scalar_add(
                out=msg_sb[:], in0=ps_msg[:], scalar1=biasp[0:MSG, 18:19])
            msg_r = msg_sb[:]
            memT_r = memf[:]

            # ---- GRU + prediction. Matmuls run at half width (f32r needs
            #      free >= 256); the serial ACT/DVE/gpsimd gate chain runs in
            #      quarter-width chunks so it pipelines deeper. ----
            ps_rz = pmm.tile([128, R], F32, tag="mm")
            nc.tensor.matmul(out=ps_rz[:], lhsT=wi_sb[:, 0:128], rhs=msg_r,
                             start=True, stop=False)
            nc.tensor.matmul(out=ps_rz[:], lhsT=wh_sb[:, 0:128], rhs=memT_r,
                             start=False, stop=True)
            ps_in = pmm.tile([MD, R], F32, tag="mm")
            nc.tensor.matmul(out=ps_in[:], lhsT=wi_sb[:, 128:192], rhs=msg_r,
                             start=True, stop=True)
            rz = gates.tile([128, R], F32, tag="rz")
            rhn = gates.tile([MD, R], F32, tag="rhn")
            npre = gates.tile([MD, R], F32, tag="npre")
            n_t = gates.tile([MD, R], F32, tag="n_t")
            d_t = gates.tile([MD, R], F32, tag="d_t")
            z_t = gates.tile([MD, R], F32, tag="z_t")
            zd = gates.tile([MD, R], F32, tag="zd")
            # actx rows 0:64 = relu(pred1); row 64 = ones so the out matmul's
            # 65th contraction row adds bp2 (folded into wp2 row 64)
            actx = gates.tile([MD + 1, R], F32R, tag="actx")
            nc.scalar.dma_start(out=actx[MD:MD + 1, :], in_=ones_d[:])
            upd = gates.tile([MD, R], F32R, tag="upd")
            ps_pred = pacc.tile([MD, R], F32, tag="pred")
            QR = R // 4
            for x in range(4):
                cs = slice(x * QR, (x + 1) * QR)
                nc.scalar.activation(rz[:, cs], ps_rz[:, cs], AF.Sigmoid,
                                     bias=biasp[:, 17:18])
                nc.vector.tensor_mul(out=rhn[:, cs], in0=rz[0:MD, cs], in1=hnb[:, cs])
                nc.vector.tensor_add(out=npre[:, cs], in0=ps_in[:, cs], in1=rhn[:, cs])
                nc.scalar.activation(n_t[:, cs], npre[:, cs], AF.Tanh,
                                     bias=biasp[0:MD, 19:20])
                nc.gpsimd.tensor_sub(out=d_t[:, cs], in0=memf[:, cs].bitcast(F32),
                                      in1=n_t[:, cs])
                nc.gpsimd.tensor_copy(out=z_t[:, cs], in_=rz[MD:128, cs])
                nc.vector.tensor_mul(out=zd[:, cs], in0=z_t[:, cs], in1=d_t[:, cs])
                nc.vector.tensor_add(out=upd[:, cs], in0=n_t[:, cs], in1=zd[:, cs])
                if x % 2 == 1:
                    hs = slice((x - 1) * QR, (x + 1) * QR)
                    nc.tensor.matmul(out=ps_pred[:, hs], lhsT=wp1_sb[:],
                                     rhs=upd[:, hs], start=True, stop=True)
                    nc.scalar.activation(actx[0:MD, hs], ps_pred[:, hs], AF.Relu,
                                         bias=biasp[0:MD, 21:22])
            act_r = actx[:]

            for m in range(32):
                ps_o = pmm.tile([128, R], F32, tag="mm")
                nc.tensor.matmul(out=ps_o[:], lhsT=wp2_sb[:, m * 128:(m + 1) * 128],
                                 rhs=act_r, start=True, stop=True)
                if m % 4 == 0:
                    stage = ostg.tile([128, 4 * R], BF16, tag="stage")
                dst = stage[:, (m % 4) * R:(m % 4 + 1) * R]
                # alternate PSUM->SBUF bf16 casts across DVE and ACT (gpsimd
                # cannot read PSUM) so neither engine serializes the tail
                if m % 2 == 0:
                    nc.vector.tensor_copy(out=dst, in_=ps_o[:])
                else:
                    nc.scalar.activation(dst, ps_o[:], AF.Copy)
                if m % 4 == 3:
                    nc.sync.dma_start(
                        out=out_d[:, (m - 3) * R:(m + 1) * R], in_=stage[:])

    nc.compile()
    return nc


def _get_program():
    global _PROG
    if _PROG is None:
        _PROG = _build_program()
    return _PROG


def _host_prep(memory, od_mat, walks, W_rw, b_rw, W1, b1, W2, b2,
               gru_Wi, gru_bi, gru_Wh, gru_bh, Wp1, bp1, Wp2, bp2):
    import ml_dtypes
    f = np.float32
    E4 = ml_dtypes.float8_e4m3
    memory = np.ascontiguousarray(np.asarray(memory), dtype=f)
    od_mat = np.asarray(od_mat)
    walks = np.asarray(walks).astype(np.int64)
    W_rw = np.asarray(W_rw, dtype=f); b_rw = np.asarray(b_rw, dtype=f)
    W1 = np.asarray(W1, dtype=f); b1 = np.asarray(b1, dtype=f)
    W2 = np.asarray(W2, dtype=f); b2 = np.asarray(b2, dtype=f)
    gru_Wi = np.asarray(gru_Wi, dtype=f); gru_bi = np.asarray(gru_bi, dtype=f)
    gru_Wh = np.asarray(gru_Wh, dtype=f); gru_bh = np.asarray(gru_bh, dtype=f)
    Wp1 = np.asarray(Wp1, dtype=f); bp1 = np.asarray(bp1, dtype=f)
    Wp2 = np.asarray(Wp2, dtype=f); bp2 = np.asarray(bp2, dtype=f)

    W1dest = W1[:, 0:MD]
    W1od = W1[:, MD:MD + N]
    W1rw = W1[:, MD + N:]
    W1g = (W1rw @ W_rw) / np.float32(8.0)

    # od k-tiles: fp8(16 * W1od.T), k-subtile-major for DoubleRow pairs
    W1odT = np.zeros((32 * 128, HIDP), dtype=f)
    W1odT[:, :HID] = W1od.T * np.float32(SC)
    w1h = np.ascontiguousarray(
        W1odT.reshape(32, 128, HT, 128)
        .transpose(2, 1, 0, 3).reshape(HT, 128, 32 * 128).astype(E4))

    # mixed k-tile weights [dest | walk], x256 to match the fp8 product scale
    W1m = np.concatenate([W1dest, W1g], axis=1) * np.float32(SC * SC)  # [2112,128]
    W1mT = np.zeros((128, HIDP), dtype=f)
    W1mT[:, :HID] = W1m.T
    w1m = np.ascontiguousarray(W1mT.astype(ml_dtypes.bfloat16))

    # b1 fold: rw bias + the exact od mean-centering rank-1 term, x256
    b1p = np.zeros(HIDP, dtype=f)
    b1p[:HID] = (b1 + W1rw @ b_rw + np.float32(0.5) * W1od.sum(axis=1)) \
        * np.float32(SC * SC)

    # W2 absorbs the 1/256 descale
    W2tp = np.zeros((HIDP, MSG), dtype=f)
    W2tp[:HID] = W2.T / np.float32(SC * SC)
    w2t = np.ascontiguousarray(
        W2tp.reshape(HT, 128, MSG).transpose(1, 0, 2).reshape(128, HT * MSG))

    # prediction: fold bp2 into wp2 as a 65th contraction row
    wp2x = np.concatenate(
        [np.ascontiguousarray(Wp2.T), bp2[None, :]], axis=0)  # [65, N]

    # fp8 memory in k-subtile-major layout (lhsT of the gather matmuls)
    mt8 = np.ascontiguousarray(
        memory.reshape(32, 128, MD).transpose(1, 0, 2)
        .reshape(128, 32 * MD).astype(E4))

    def pad128(v):
        o = np.zeros(128, dtype=f)
        o[:v.shape[0]] = v
        return o

    # biases packed as [128 partitions, 64 columns]
    biases = np.zeros((64, 128), dtype=f)
    biases[0:HT] = b1p.reshape(HT, 128)
    biases[17] = gru_bi[:128] + gru_bh[:128]
    biases[18] = pad128(b2)
    biases[19] = pad128(gru_bi[128:])
    biases[20] = pad128(gru_bh[128:])
    biases[21] = pad128(bp1)
    biases = np.ascontiguousarray(biases.T)                    # [128, 64]

    shared = {
        "w1h": w1h,
        "w1m": w1m,
        "w2t": w2t,
        "mt8": mt8,
        "wi": np.ascontiguousarray(gru_Wi.T),
        "wh": np.ascontiguousarray(gru_Wh.T),
        "wp1": np.ascontiguousarray(Wp1.T),
        "wp2": np.ascontiguousarray(wp2x),
        "biases": biases,
        "ones": np.ones((1, R), dtype=f),
    }
    cols = np.tile(np.arange(R, dtype=np.int64)[:, None], (1, WL)).ravel()
    in_maps = []
    for c in range(NC):
        sl = slice(c * R, (c + 1) * R)
        odc = np.asarray(od_mat[sl], dtype=f)
        # odv[p, k*R+n] = 16*(od[c*R+n, k*128+p] - 0.5), fp8
        odv = np.ascontiguousarray(
            ((odc.T - np.float32(0.5)) * np.float32(SC))
            .reshape(32, 128, R).transpose(1, 0, 2).reshape(128, 32 * R)
            .astype(E4))
        # walk count matrix A[m, n] = #{j: walks[c*R+n, j] = m}, fp8 exact
        A = np.zeros((N, R), dtype=f)
        np.add.at(A, (walks[sl].ravel(), cols), np.float32(1.0))
        aw = np.ascontiguousarray(
            A.reshape(32, 128, R).transpose(1, 0, 2).reshape(128, 32 * R)
            .astype(E4))
        memT = np.ascontiguousarray(memory[sl].T)
        in_maps.append(dict(
            shared,
            memT=np.ascontiguousarray(memT.astype(ml_dtypes.bfloat16)),
            memf=memT,
            odv=odv,
            aw=aw,
        ))
    return in_maps


def _assemble(results):
    od = np.empty((N, N), dtype=np.float32)
    for c in range(NC):
        # outm[p, m*R+n] = od[c*R+n, m*128+p]
        od[c * R:(c + 1) * R, :] = (
            results[c]["outm"].astype(np.float32)
            .reshape(128, 32, R).transpose(2, 1, 0).reshape(R, N))
    return od


def _install_ntff_shim():
    """The agent image's antenv lacks axon_hooks, so trace=True dies on
    import. Recreate the module with the ctypes-based NTFF hook that
    trn_agent_boot would have registered."""
    import sys
    import types
    if "antenv.axon_hooks" in sys.modules:
        return
    from trn_agent_boot.trn_boot import _ntff_profile_via_ctypes
    hook = _ntff_profile_via_ctypes("/opt/axon/libaxon_pjrt.so")
    mod = types.ModuleType("antenv.axon_hooks")
    mod._hook = hook
    mod.get_axon_ntff_profile_hook = lambda: mod._hook
    mod.set_axon_ntff_profile_hook = lambda h: setattr(mod, "_hook", h)
    sys.modules["antenv.axon_hooks"] = mod


def run(inputs, trace=False):
    """Run on 8 NeuronCores; returns (od [N,N] f32, BassKernelResults)."""
    from concourse.bass_utils import run_bass_kernel_spmd
    if trace:
        try:
            _install_ntff_shim()
        except Exception as e:
            print(f"ntff shim failed ({e}); running without trace")
            trace = False
    nc = _get_program()
    in_maps = _host_prep(**inputs)
    res = run_bass_kernel_spmd(nc, in_maps, list(range(NC)), trace=trace)
    return _assemble(res.results), res


def kernel(**inputs):
    od, _ = run(inputs)
    return od
